# revision 1
# baseline (speedup 1.0000x reference)
"""Multi-head causal attention kernel for Trainium2 (8 NeuronCores).

Problem: B=4, S=2048, HID=1024, H=16 heads (head_dim 64), causal mask,
fp32 I/O.  out = softmax(mask + (XqWq)(XkWk)^T/8) (XvWv) Wo

Sharding: 8 cores = 4 batches x 2 head-groups.  Core c handles batch
c//2 and heads (c%2)*8 .. +8 (dk slice of 512).  Each core computes a
full-shape [S, HID] partial output (its head-group's contribution
through Wo); the host sums the two partials per batch.

Per-core dataflow (all matmuls in float32r = TF32-like, full PE rate):
  - PE-transpose X chunks -> X^T; project to kT (persistent, [e,s]
    layout, 2 heads per 128-partition tile), qT (rotating per-512-q
    window) and v (persistent, natural [s,e] with a ones column per
    head so the PV matmul also emits softmax denominators).
  - Attention in transposed [k,q] orientation per (q-window j, head
    pair): logits^T = kT-chunk (stationary) x qT (moving) with
    causally-restricted columns; additive triangular mask on diagonal
    blocks (DVE, in PSUM); exp on ScalarE PSUM->SBUF; PV accumulates
    ctx^T in PSUM (per-element has_written makes partial-range
    accumulation correct).  Denominator row -> reciprocal (DVE) ->
    partition_broadcast (GpSimd) -> multiply-evacuate ctx^T (DVE).
  - Output projection ctx^T.T @ Wo per q-window, fused into the stream.

The projection work for q-window j+1 is emitted interleaved with the
attention work of window j: the projection matmuls act as PE filler
that keeps the PE HAM activity monitor busy (otherwise the exp-bound
attention inner loop lets the PE clock-gate down to 1.2 GHz).
"""

import numpy as np

B, S, HID = 4, 2048, 1024
H_LOCAL, E_LOCAL = 8, 512  # heads / dk columns handled per core
N_CORES = 8
USE_F32R = True

_cached = {}


def _build():
    from concourse import bacc
    import concourse.bass as bass
    import concourse.mybir as mybir
    import concourse.tile as tile
    from concourse.masks import make_identity

    F32 = mybir.dt.float32
    F32R = mybir.dt.float32r if USE_F32R else mybir.dt.float32
    Exp = mybir.ActivationFunctionType.Exp

    nc = bacc.Bacc()
    xq = nc.dram_tensor("xq", [S, HID], F32R, kind="ExternalInput")
    xk = nc.dram_tensor("xk", [S, HID], F32R, kind="ExternalInput")
    xv = nc.dram_tensor("xv", [S, HID], F32R, kind="ExternalInput")
    wq = nc.dram_tensor("wq", [HID, E_LOCAL], F32R, kind="ExternalInput")
    wk = nc.dram_tensor("wk", [HID, E_LOCAL], F32R, kind="ExternalInput")
    wv = nc.dram_tensor("wv", [HID, E_LOCAL], F32R, kind="ExternalInput")
    wo = nc.dram_tensor("wo", [E_LOCAL, HID], F32R, kind="ExternalInput")
    out = nc.dram_tensor("out", [S, HID], F32, kind="ExternalOutput")

    NST = 8           # projection s-tiles
    STW = S // NST    # 256 rows per s-tile
    NSC = STW // 128  # 2 s-chunks per s-tile
    NDC = HID // 128  # 8 d-chunks
    NEC = E_LOCAL // 128  # 4 e-chunks = head pairs
    NKC = S // 128    # 16 k-chunks
    NQT = 4           # q windows of 512

    with tile.TileContext(nc) as tc:
        with (
            tc.sbuf_pool(name="consts", bufs=1) as consts,
            tc.sbuf_pool(name="persist", bufs=1) as persist,
            tc.sbuf_pool(name="stream", bufs=1) as sm,
            tc.psum_pool(name="ps", bufs=1) as ps,
        ):
            ident_f = consts.tile([128, 128], F32)
            make_identity(nc, ident_f)
            ident = consts.tile([128, 128], F32R)
            nc.vector.tensor_copy(ident, ident_f)
            # additive causal mask for diagonal [k,q] blocks: 0 where
            # k <= q else -1e9
            trimask = consts.tile([128, 128], F32)
            nc.gpsimd.memset(trimask, 0.0)
            nc.gpsimd.affine_select(
                out=trimask, in_=trimask,
                compare_op=mybir.AluOpType.is_ge, fill=-1e9, base=0,
                pattern=[[1, 128]], channel_multiplier=-1,
            )
            ones_col = consts.tile([128, 1], F32)
            nc.vector.memset(ones_col, 1.0)

            kt_sb = [persist.tile([128, S], F32R, name=f"kt{i}", tag=f"kt{i}")
                     for i in range(NEC)]
            v_sb = [persist.tile([128, H_LOCAL, 65], F32R, name=f"v{i}",
                                 tag=f"v{i}") for i in range(NKC)]

            wq_sb = sm.tile([128, NDC, E_LOCAL], F32R, tag="wq", bufs=1)
            wk_sb = sm.tile([128, NDC, E_LOCAL], F32R, tag="wk", bufs=1)
            wv_sb = sm.tile([128, NDC, E_LOCAL], F32R, tag="wv", bufs=1)
            wo_sb = sm.tile([128, NEC, HID], F32R, tag="wo", bufs=1)
            nc.sync.dma_start(
                out=wq_sb, in_=wq.rearrange("(dc p) e -> p dc e", p=128))
            nc.sync.dma_start(
                out=wk_sb, in_=wk.rearrange("(dc p) e -> p dc e", p=128))
            nc.sync.dma_start(
                out=wv_sb, in_=wv.rearrange("(dc p) e -> p dc e", p=128))
            nc.sync.dma_start(
                out=wo_sb, in_=wo.rearrange("(dv p) n -> p dv n", p=128))

            qt_rot = {}   # (window, ec) -> [128, 512] tile
            ctx_rot = {}  # (window, hp) -> [128, 512] tile

            def proj_unit(st, tname):
                """Load + transpose + project one input tensor s-tile."""
                s0 = st * STW
                w = st // 2
                xdram = {"q": xq, "k": xk, "v": xv}[tname]
                xnat = sm.tile([128, NSC, HID], F32R, tag="xnat", bufs=2,
                               name=f"xnat_{tname}{st}")
                nc.sync.dma_start(
                    out=xnat,
                    in_=xdram[s0:s0 + STW, :].rearrange(
                        "(sc p) d -> p sc d", p=128))
                xt = sm.tile([128, NDC, STW], F32R, tag="xt", bufs=2,
                             name=f"xt_{tname}{st}")
                for dcp in range(NDC // 2):
                    tp = ps.tile([128, 512], F32R, tag="work", bufs=2,
                                 name=f"tp_{tname}{st}_{dcp}")
                    for k2 in range(2):
                        dc = dcp * 2 + k2
                        for sc in range(NSC):
                            nc.tensor.transpose(
                                tp[:, k2 * STW + sc * 128:
                                   k2 * STW + (sc + 1) * 128],
                                xnat[:, sc, dc * 128:(dc + 1) * 128],
                                ident)
                    nc.vector.tensor_copy(xt[:, dcp * 2:dcp * 2 + 2, :], tp)

                if tname == "q":
                    for ec in range(NEC):
                        if st % 2 == 0:
                            qt_rot[(w, ec)] = sm.tile(
                                [128, 512], F32R, tag=f"qtr{ec}", bufs=2,
                                name=f"qtr{ec}_{w}")
                        pj = ps.tile([128, STW], F32, tag="work", bufs=2,
                                     name=f"pjq_{st}_{ec}")
                        for dc in range(NDC):
                            nc.tensor.matmul(
                                pj, wq_sb[:, dc, ec * 128:(ec + 1) * 128],
                                xt[:, dc, :],
                                start=(dc == 0), stop=(dc == NDC - 1))
                        off = (st % 2) * STW
                        nc.vector.tensor_copy(
                            qt_rot[(w, ec)][:, off:off + STW], pj)
                elif tname == "k":
                    for ec in range(NEC):
                        pj = ps.tile([128, STW], F32, tag="work", bufs=2,
                                     name=f"pjk_{st}_{ec}")
                        for dc in range(NDC):
                            nc.tensor.matmul(
                                pj, wk_sb[:, dc, ec * 128:(ec + 1) * 128],
                                xt[:, dc, :],
                                start=(dc == 0), stop=(dc == NDC - 1))
                        nc.vector.tensor_copy(
                            kt_sb[ec][:, s0:s0 + STW], pj)
                else:
                    for sc in range(NSC):
                        pv = ps.tile([128, E_LOCAL], F32, tag="work", bufs=2,
                                     name=f"pv_{st}_{sc}")
                        for dc in range(NDC):
                            nc.tensor.matmul(
                                pv, xt[:, dc, sc * 128:(sc + 1) * 128],
                                wv_sb[:, dc, :],
                                start=(dc == 0), stop=(dc == NDC - 1))
                        ci = st * NSC + sc
                        nc.vector.tensor_copy(
                            v_sb[ci][:, :, 0:64],
                            pv.rearrange("p (h e) -> p h e", h=H_LOCAL))
                        ones_b = bass.AP(
                            tensor=ones_col.tensor, offset=ones_col.offset,
                            ap=[ones_col.ap[0], [0, H_LOCAL],
                                ones_col.ap[1]],
                        )
                        nc.vector.tensor_copy(v_sb[ci][:, :, 64:65], ones_b)

            def attention_unit(j, hp):
                q0 = j * 512
                nlast = 4 * j + 3
                qt = qt_rot[(j, hp)]
                cpx = [ps.tile([65, 512], F32, tag="cpx", bufs=2,
                               name=f"cpx{hp}_{j}_{hi}") for hi in range(2)]
                ctx_rot[(j, hp)] = sm.tile([128, 512], F32R, tag=f"ctxr{hp}",
                                           bufs=2, name=f"ctxr{hp}_{j}")
                for c in range(4 * j + 4):
                    vo = max(0, c * 128 - q0)
                    lg = ps.tile([128, 1024], F32, tag="lg", bufs=2,
                                 name=f"lg{hp}_{j}_{c}")
                    pt = sm.tile([128, 1024], F32R, tag="pt", bufs=2,
                                 name=f"pt{hp}_{j}_{c}")
                    for hi in range(2):
                        nc.tensor.matmul(
                            lg[:, hi * 512 + vo:(hi + 1) * 512],
                            kt_sb[hp][hi * 64:(hi + 1) * 64,
                                      c * 128:(c + 1) * 128],
                            qt[hi * 64:(hi + 1) * 64, vo:512],
                            start=True, stop=True)
                    if c >= 4 * j:
                        m = c - 4 * j
                        blk = lg.rearrange("p (hh q) -> p hh q", hh=2)[
                            :, :, m * 128:(m + 1) * 128]
                        tri_b = bass.AP(
                            tensor=trimask.tensor, offset=trimask.offset,
                            ap=[trimask.ap[0], [0, 2], trimask.ap[1]],
                        )
                        nc.vector.tensor_add(blk, blk, tri_b)
                    nc.scalar.activation(pt[:, vo:1024], lg[:, vo:1024], Exp)
                    for hi in range(2):
                        nc.tensor.matmul(
                            cpx[hi][:, vo:512],
                            v_sb[c][:, hp * 2 + hi, :],
                            pt[:, hi * 512 + vo:(hi + 1) * 512],
                            start=(c == 0), stop=(c == nlast))
                for hi in range(2):
                    bcast = sm.tile([64, 512], F32, tag="bcast", bufs=1,
                                    name=f"bc{hp}_{j}_{hi}")
                    nc.vector.tensor_copy(bcast[0:1, :], cpx[hi][64:65, :])
                    nc.vector.reciprocal_approx_fast(
                        out=bcast[0:1, :], in_=bcast[0:1, :])
                    nc.gpsimd.partition_broadcast(bcast, bcast[0:1, :])
                    nc.vector.tensor_mul(
                        ctx_rot[(j, hp)][hi * 64:(hi + 1) * 64, :],
                        cpx[hi][0:64, :], bcast)

            for st in (0, 1):
                for t in ("q", "k", "v"):
                    proj_unit(st, t)

            # Emission = program order: every producer must be emitted
            # before its consumers.  Projection for window j+1 is emitted
            # interleaved with attention(j) as PE filler; attention(3)
            # (the largest window, no projection left) gets the deferred
            # out-projection of window 2 as filler instead.
            fills = {
                0: [(2, "q"), (2, "k"), (2, "v"), (3, "q"), (3, "k"),
                    (3, "v")],
                1: [(4, "q"), (4, "k"), (4, "v"), (5, "q"), (5, "k"),
                    (5, "v")],
                2: [(6, "q"), (6, "k"), (6, "v"), (7, "q"), (7, "k"),
                    (7, "v")],
                3: [],
            }

            def out_block(qc):
                for nh in range(2):
                    po = ps.tile([128, 512], F32, tag="work", bufs=2,
                                 name=f"po{qc}_{nh}")
                    for dvc in range(NEC):
                        nc.tensor.matmul(
                            po,
                            ctx_rot[(qc // 4, dvc)][:,
                                                    (qc % 4) * 128:
                                                    (qc % 4 + 1) * 128],
                            wo_sb[:, dvc, nh * 512:(nh + 1) * 512],
                            start=(dvc == 0), stop=(dvc == NEC - 1))
                    osb = sm.tile([128, 512], F32, tag="osb", bufs=1,
                                  name=f"osb{qc}_{nh}")
                    nc.scalar.copy(osb, po)
                    nc.sync.dma_start(
                        out=out[qc * 128:(qc + 1) * 128,
                                nh * 512:(nh + 1) * 512],
                        in_=osb)

            for j in range(3):
                fill = list(fills[j])
                for hp in range(NEC):
                    attention_unit(j, hp)
                    for _ in range(2):
                        if fill:
                            proj_unit(*fill.pop(0))
                while fill:
                    proj_unit(*fill.pop(0))
                if j < 2:
                    for qc in range(4 * j, 4 * j + 4):
                        out_block(qc)
            # j = 3: out(2) blocks act as the PE filler
            for hp in range(NEC):
                attention_unit(3, hp)
                out_block(8 + hp)
            for qc in range(12, 16):
                out_block(qc)

    nc.compile()
    return nc


def kernel(queries, keys, values, mask=None, Wq=None, Wk=None, Wv=None,
           Wo=None, **_ignored):
    from concourse.bass_utils import run_bass_kernel_spmd

    if "nc" not in _cached:
        _cached["nc"] = _build()
    nc = _cached["nc"]

    scale = np.float32(0.125)  # (DK//H) ** -0.5, exact power of two
    in_maps = []
    for c in range(N_CORES):
        b, g = divmod(c, 2)
        sl = slice(g * E_LOCAL, (g + 1) * E_LOCAL)
        in_maps.append({
            "xq": np.ascontiguousarray(queries[b], dtype=np.float32),
            "xk": np.ascontiguousarray(keys[b], dtype=np.float32),
            "xv": np.ascontiguousarray(values[b], dtype=np.float32),
            "wq": np.ascontiguousarray(Wq[:, sl] * scale),
            "wk": np.ascontiguousarray(Wk[:, sl]),
            "wv": np.ascontiguousarray(Wv[:, sl]),
            "wo": np.ascontiguousarray(Wo[sl, :]),
        })
    res = run_bass_kernel_spmd(nc, in_maps, core_ids=list(range(N_CORES)))
    outs = res.results
    full = np.empty((B, S, HID), np.float32)
    for b in range(B):
        full[b] = outs[2 * b]["out"] + outs[2 * b + 1]["out"]
    return full



# revision 18
# speedup vs baseline: 1.3394x; 1.3394x over previous
"""Multi-head causal attention kernel for Trainium2 (8 NeuronCores).

Problem: B=4, S=2048, HID=1024, H=16 heads (head_dim 64), causal mask,
fp32 I/O.  out = softmax(mask + (XqWq)(XkWk)^T/8) (XvWv) Wo

Sharding: 8 cores = 4 batches x 2 head-groups.  Core c handles batch
c//2 and heads (c%2)*8 .. +8 (dk slice of 512).  Each core computes a
full-shape [S, HID] partial output (its head-group's contribution
through Wo); the host sums the two partials per batch.

v2 design (vs the PE-transpose baseline):
  - Host pre-transposes X to X^T [HID, S] and casts all matmul operands
    to bf16: no PE transposes (was 384 matmuls + 96 DVE evacuations),
    half the input DMA bytes, and bf16 avoids fp32r's small-N matmul
    penalty on the causal-diagonal blocks.
  - Projections read X^T s-blocks of 512 straight from DRAM:
    q^T/k^T [e,s] via W-chunk stationary x X^T moving; v [s,e] natural
    via X^T-chunk stationary x W moving (with a ones column per head so
    the PV matmul also emits softmax denominators).
  - Attention per (512-q window j, head-pair hp) in [k,q] orientation:
    logits^T in PSUM, one strided exp per k-chunk covering both heads'
    valid columns only (no junk columns), causal mask applied as a
    post-exp multiplicative 0/1 triangle on bf16 SBUF (4x DVE mode)
    instead of an additive -1e9 on PSUM.
  - Normalize: reciprocal straight off the PSUM denominator row,
    GpSimd partition_broadcast, one DVE multiply into bf16 ctx^T.
  - Output projection ctx^T.T @ Wo per q-chunk, DMA'd to DRAM directly
    from PSUM (no staging copy).
  - Projection/out-proj matmuls interleave with the exp-bound attention
    stream as PE filler (keeps the PE HAM activity monitor warm).
"""

import numpy as np

B, S, HID = 4, 2048, 1024
H_LOCAL, E_LOCAL = 8, 512  # heads / dk columns handled per core
N_CORES = 8

_cached = {}


def _salt_width():
    """Hash of this file's source, used to perturb an (unused) input
    tensor's shape.  The Neuron compile cache keys on the HLO module,
    which does NOT include the bass BIR payload (it rides in the
    custom_call backend_config) — without this, two different kernel
    builds with identical I/O signatures collide on one cache entry and
    a stale NEFF gets executed."""
    import hashlib

    with open(__file__, "rb") as f:
        h = hashlib.sha256(f.read()).hexdigest()
    return 2 + int(h, 16) % 4093


def _build():
    from concourse import bacc
    import concourse.bass as bass
    import concourse.mybir as mybir
    import concourse.tile as tile

    F32 = mybir.dt.float32
    BF16 = mybir.dt.bfloat16
    Exp = mybir.ActivationFunctionType.Exp

    nc = bacc.Bacc()
    nc.dram_tensor("salt", [1, _salt_width()], mybir.dt.uint8,
                   kind="ExternalInput")
    xq = nc.dram_tensor("xq", [HID, S], BF16, kind="ExternalInput")
    xk = nc.dram_tensor("xk", [HID, S], BF16, kind="ExternalInput")
    xv = nc.dram_tensor("xv", [HID, S], BF16, kind="ExternalInput")
    wq = nc.dram_tensor("wq", [HID, E_LOCAL], BF16, kind="ExternalInput")
    wk = nc.dram_tensor("wk", [HID, E_LOCAL], BF16, kind="ExternalInput")
    wv = nc.dram_tensor("wv", [HID, E_LOCAL], BF16, kind="ExternalInput")
    wo = nc.dram_tensor("wo", [E_LOCAL, HID], BF16, kind="ExternalInput")
    out = nc.dram_tensor("out", [S, HID], F32, kind="ExternalOutput")

    NDC = HID // 128       # 8 d-chunks
    NEC = E_LOCAL // 128   # 4 e-chunks = head pairs
    NKC = S // 128         # 16 k-chunks
    NSB = 4                # s-blocks of 512 (== q windows)
    SBW = S // NSB         # 512

    with tile.TileContext(nc) as tc:
        with (
            tc.sbuf_pool(name="consts", bufs=1) as consts,
            tc.sbuf_pool(name="persist", bufs=1) as persist,
            tc.sbuf_pool(name="stream", bufs=1) as sm,
            tc.psum_pool(name="ps", bufs=1) as ps,
        ):
            # multiplicative causal mask for diagonal [k,q] blocks:
            # 1 where k <= q else 0
            trif = consts.tile([128, 128], F32)
            nc.gpsimd.memset(trif, 1.0)
            nc.gpsimd.affine_select(
                out=trif, in_=trif,
                compare_op=mybir.AluOpType.is_ge, fill=0.0, base=0,
                pattern=[[1, 128]], channel_multiplier=-1,
            )
            trimask = consts.tile([128, 128], BF16)
            nc.vector.tensor_copy(trimask, trif)
            # warm the exp table while initial DMAs run
            warm = consts.tile([1, 2], F32)
            nc.vector.memset(warm, 0.0)
            nc.scalar.activation(warm, warm, Exp)

            kt_sb = [persist.tile([128, S], BF16, name=f"kt{i}", tag=f"kt{i}")
                     for i in range(NEC)]
            v_sb = [persist.tile([128, H_LOCAL, 65], BF16, name=f"v{i}",
                                 tag=f"v{i}") for i in range(NKC)]
            for i in range(NKC):
                nc.vector.memset(v_sb[i][:, :, 64:65], 1.0)

            wq_sb = sm.tile([128, NDC, E_LOCAL], BF16, tag="wq", bufs=1)
            wk_sb = sm.tile([128, NDC, E_LOCAL], BF16, tag="wk", bufs=1)
            wv_sb = sm.tile([128, NDC, E_LOCAL], BF16, tag="wv", bufs=1)
            wo_sb = sm.tile([128, NEC, HID], BF16, tag="wo", bufs=1)
            w_dram = {"q": wq, "k": wk, "v": wv}
            w_sb = {"q": wq_sb, "k": wk_sb, "v": wv_sb}

            qt_rot = {}   # (window, ec) -> [128, 512] bf16 q^T tile
            ctx_rot = {}  # (window, hp) -> [128, 512] bf16 ctx^T tile

            def proj_unit(sb, tname, split_dma=False):
                """Load one X^T s-block and project one tensor."""
                s0 = sb * SBW
                xdram = {"q": xq, "k": xk, "v": xv}[tname]
                xt = sm.tile([128, NDC, SBW], BF16, tag="xt", bufs=2,
                             name=f"xt_{tname}{sb}")
                xsrc = xdram[:, s0:s0 + SBW].rearrange(
                    "(dc p) s -> p dc s", p=128)
                if split_dma:
                    h = NDC // 2
                    nc.sync.dma_start(out=xt[:, 0:h, :], in_=xsrc[:, 0:h, :])
                    nc.sync.dma_start(out=xt[:, h:NDC, :],
                                      in_=xsrc[:, h:NDC, :])
                else:
                    nc.sync.dma_start(out=xt, in_=xsrc)
                if tname == "q":
                    for ec in range(NEC):
                        qt_rot[(sb, ec)] = sm.tile(
                            [128, SBW], BF16, tag=f"qtr{ec}", bufs=2,
                            name=f"qtr{ec}_{sb}")
                        pj = ps.tile([128, SBW], F32, tag="work", bufs=2,
                                     name=f"pjq_{sb}_{ec}")
                        for dc in range(NDC):
                            nc.tensor.matmul(
                                pj, wq_sb[:, dc, ec * 128:(ec + 1) * 128],
                                xt[:, dc, :],
                                start=(dc == 0), stop=(dc == NDC - 1))
                        nc.vector.tensor_copy(qt_rot[(sb, ec)], pj)
                elif tname == "k":
                    for ec in range(NEC):
                        pj = ps.tile([128, SBW], F32, tag="work", bufs=2,
                                     name=f"pjk_{sb}_{ec}")
                        for dc in range(NDC):
                            nc.tensor.matmul(
                                pj, wk_sb[:, dc, ec * 128:(ec + 1) * 128],
                                xt[:, dc, :],
                                start=(dc == 0), stop=(dc == NDC - 1))
                        nc.vector.tensor_copy(
                            kt_sb[ec][:, s0:s0 + SBW], pj)
                else:
                    for sc in range(SBW // 128):
                        pv = ps.tile([128, E_LOCAL], F32, tag="work", bufs=2,
                                     name=f"pv_{sb}_{sc}")
                        for dc in range(NDC):
                            nc.tensor.matmul(
                                pv, xt[:, dc, sc * 128:(sc + 1) * 128],
                                wv_sb[:, dc, :],
                                start=(dc == 0), stop=(dc == NDC - 1))
                        ci = sb * (SBW // 128) + sc
                        nc.vector.tensor_copy(
                            v_sb[ci][:, :, 0:64],
                            pv.rearrange("p (h e) -> p h e", h=H_LOCAL))

            def attention_unit(j, hp):
                q0 = j * SBW
                nlast = 4 * j + 3
                qt = qt_rot[(j, hp)]
                cpx = [ps.tile([65, SBW], F32, tag="cpx", bufs=2,
                               name=f"cpx{hp}_{j}_{hi}") for hi in range(2)]
                ctx_rot[(j, hp)] = sm.tile([128, SBW], BF16, tag=f"ctxr{hp}",
                                           bufs=2, name=f"ctxr{hp}_{j}")

                def emit_pv(c, vo, pt, his=(0, 1)):
                    for hi in his:
                        nc.tensor.matmul(
                            cpx[hi][:, vo:512],
                            v_sb[c][:, hp * 2 + hi, :],
                            pt[:, hi, vo:512],
                            start=(c == 0), stop=(c == nlast))

                def normalize(hi):
                    # copy-then-reciprocal: DVE reciprocal mis-executes on
                    # HW when given a cross-partition-shift PSUM source
                    # (CoreSim accepts it), so keep the recip in SBUF with
                    # no partition shift
                    bcast = sm.tile([64, SBW], F32, tag="bcast", bufs=2,
                                    name=f"bc{hp}_{j}_{hi}")
                    nc.vector.tensor_copy(bcast[0:1, :], cpx[hi][64:65, :])
                    nc.vector.reciprocal_approx_fast(
                        out=bcast[0:1, :], in_=bcast[0:1, :])
                    nc.gpsimd.partition_broadcast(bcast, bcast[0:1, :])
                    nc.vector.tensor_mul(
                        ctx_rot[(j, hp)][hi * 64:(hi + 1) * 64, :],
                        cpx[hi][0:64, :], bcast)

                # software-pipelined: logits(c+1) is emitted before PV(c)
                # so the PE has queued matmuls while ACT runs exp(c)
                pending = None
                for c in range(4 * j + 4):
                    vo = max(0, c * 128 - q0)
                    lg = ps.tile([128, 1024], F32, tag="lg", bufs=2,
                                 name=f"lg{hp}_{j}_{c}")
                    pt = sm.tile([128, 2, SBW], BF16, tag="pt", bufs=2,
                                 name=f"pt{hp}_{j}_{c}")
                    for hi in range(2):
                        nc.tensor.matmul(
                            lg[:, hi * 512 + vo:(hi + 1) * 512],
                            kt_sb[hp][hi * 64:(hi + 1) * 64,
                                      c * 128:(c + 1) * 128],
                            qt[hi * 64:(hi + 1) * 64, vo:512],
                            start=True, stop=True)
                    lg3 = lg.rearrange("p (hh q) -> p hh q", hh=2)
                    nc.scalar.activation(
                        pt[:, :, vo:512], lg3[:, :, vo:512], Exp)
                    if c >= 4 * j:
                        m = c - 4 * j
                        blk = pt[:, :, m * 128:(m + 1) * 128]
                        tri_b = bass.AP(
                            tensor=trimask.tensor, offset=trimask.offset,
                            ap=[trimask.ap[0], [0, 2], trimask.ap[1]],
                        )
                        nc.vector.tensor_mul(blk, blk, tri_b)
                    if pending is not None:
                        emit_pv(*pending)
                    pending = (c, vo, pt)
                # last chunk: hi=0's normalize chain starts on DVE/GpSimd
                # while the PE still streams hi=1's final PV
                emit_pv(*pending, his=(0,))
                normalize(0)
                emit_pv(*pending, his=(1,))
                normalize(1)

            def out_block(qc):
                j, m = qc // 4, qc % 4
                for nh in range(2):
                    po = ps.tile([128, 512], F32, tag="work", bufs=2,
                                 name=f"po{qc}_{nh}")
                    for dv in range(NEC):
                        nc.tensor.matmul(
                            po,
                            ctx_rot[(j, dv)][:, m * 128:(m + 1) * 128],
                            wo_sb[:, dv, nh * 512:(nh + 1) * 512],
                            start=(dv == 0), stop=(dv == NEC - 1))
                    osb = sm.tile([128, 512], F32, tag="osb", bufs=2,
                                  name=f"osb{qc}_{nh}")
                    nc.scalar.copy(osb, po)
                    nc.sync.dma_start(
                        out=out[qc * 128:(qc + 1) * 128,
                                nh * 512:(nh + 1) * 512],
                        in_=osb)

            def out_block_tail(qc, copy_eng):
                """Full-width out-proj in the (free by now) lg PSUM pool;
                staging copies alternate DVE/ACT so they pipeline."""
                j, m = qc // 4, qc % 4
                po = ps.tile([128, 1024], F32, tag="lg", bufs=2,
                             name=f"po{qc}")
                for nh in range(2):
                    for dv in range(NEC):
                        nc.tensor.matmul(
                            po[:, nh * 512:(nh + 1) * 512],
                            ctx_rot[(j, dv)][:, m * 128:(m + 1) * 128],
                            wo_sb[:, dv, nh * 512:(nh + 1) * 512],
                            start=(dv == 0), stop=(dv == NEC - 1))
                osb = sm.tile([128, 1024], F32, tag="osbt", bufs=2,
                              name=f"osbt{qc}")
                if copy_eng == "act":
                    nc.scalar.copy(osb, po)
                else:
                    nc.vector.tensor_copy(osb, po)
                nc.sync.dma_start(
                    out=out[qc * 128:(qc + 1) * 128, :], in_=osb)

            # weight DMAs interleave with first-block projections in
            # consumption order, split in halves, so the first matmul
            # starts as soon as the first half-weight + half-X^T land
            for t in ("q", "k", "v"):
                wsrc = w_dram[t].rearrange("(dc p) e -> p dc e", p=128)
                h = NDC // 2
                nc.sync.dma_start(out=w_sb[t][:, 0:h, :],
                                  in_=wsrc[:, 0:h, :])
                nc.sync.dma_start(out=w_sb[t][:, h:NDC, :],
                                  in_=wsrc[:, h:NDC, :])
                proj_unit(0, t, split_dma=True)
            for t in ("q", "k", "v"):
                proj_unit(1, t)
            nc.sync.dma_start(
                out=wo_sb, in_=wo.rearrange("(dv p) n -> p dv n", p=128))

            # Emission = program order.  Remaining projections and the
            # out-projections of finished windows are spread one unit per
            # head-pair boundary so the PE always has queued matmuls while
            # the normalize chain runs on DVE/GpSimd.  Out-blocks of
            # window j must finish during window j+1 (their ctx tiles
            # rotate with bufs=2).
            fills = {
                (0, 0): [("p", 2, "q")], (0, 1): [("p", 2, "k")],
                (0, 2): [("p", 2, "v")], (0, 3): [("p", 3, "q")],
                (1, 0): [("o", 0)], (1, 1): [("o", 1)],
                (1, 2): [("o", 2)], (1, 3): [("o", 3), ("p", 3, "k")],
                (2, 0): [("o", 4)], (2, 1): [("o", 5)],
                (2, 2): [("o", 6)], (2, 3): [("o", 7), ("p", 3, "v")],
                (3, 0): [("o", 8)], (3, 1): [("o", 9)],
                (3, 2): [("o", 10)], (3, 3): [("o", 11)],
            }

            def emit_fill(item):
                if item[0] == "p":
                    proj_unit(item[1], item[2])
                else:
                    out_block(item[1])

            for j in range(4):
                for hp in range(NEC):
                    attention_unit(j, hp)
                    for item in fills[(j, hp)]:
                        emit_fill(item)
            for i, qc in enumerate(range(12, 16)):
                out_block_tail(qc, "act" if i % 2 else "dve")

    nc.compile()
    return nc


def _make_in_maps(queries, keys, values, Wq, Wk, Wv, Wo):
    import ml_dtypes

    bf16 = ml_dtypes.bfloat16
    scale = np.float32(0.125)  # (DK//H) ** -0.5, exact power of two
    xt = {}
    for b in range(B):
        xt[b] = (
            np.ascontiguousarray(queries[b].T).astype(bf16),
            np.ascontiguousarray(keys[b].T).astype(bf16),
            np.ascontiguousarray(values[b].T).astype(bf16),
        )
    salt = np.zeros((1, _salt_width()), np.uint8)
    in_maps = []
    for c in range(N_CORES):
        b, g = divmod(c, 2)
        sl = slice(g * E_LOCAL, (g + 1) * E_LOCAL)
        in_maps.append({
            "salt": salt,
            "xq": xt[b][0],
            "xk": xt[b][1],
            "xv": xt[b][2],
            "wq": np.ascontiguousarray(Wq[:, sl] * scale).astype(bf16),
            "wk": np.ascontiguousarray(Wk[:, sl]).astype(bf16),
            "wv": np.ascontiguousarray(Wv[:, sl]).astype(bf16),
            "wo": np.ascontiguousarray(Wo[sl, :]).astype(bf16),
        })
    return in_maps


def kernel(queries, keys, values, mask=None, Wq=None, Wk=None, Wv=None,
           Wo=None, **_ignored):
    from concourse.bass_utils import run_bass_kernel_spmd

    if "nc" not in _cached:
        _cached["nc"] = _build()
    nc = _cached["nc"]

    queries = np.asarray(queries, dtype=np.float32)
    keys = np.asarray(keys, dtype=np.float32)
    values = np.asarray(values, dtype=np.float32)
    in_maps = _make_in_maps(queries, keys, values,
                            np.asarray(Wq, dtype=np.float32),
                            np.asarray(Wk, dtype=np.float32),
                            np.asarray(Wv, dtype=np.float32),
                            np.asarray(Wo, dtype=np.float32))
    res = run_bass_kernel_spmd(nc, in_maps, core_ids=list(range(N_CORES)))
    outs = res.results
    full = np.empty((B, S, HID), np.float32)
    for b in range(B):
        full[b] = outs[2 * b]["out"] + outs[2 * b + 1]["out"]
    return full


# revision 55
# speedup vs baseline: 1.6797x; 1.2541x over previous
"""Multi-head causal attention kernel for Trainium2 (8 NeuronCores).

Problem: B=4, S=2048, HID=1024, H=16 heads (head_dim 64), causal mask,
fp32 I/O.  out = softmax(mask + (XqWq)(XkWk)^T/8) (XvWv) Wo

Sharding: 8 cores = 4 batches x 2 head-groups.  Core c handles batch
c//2 and heads (c%2)*8 .. +8 (dk slice of 512).  Each core computes a
full-shape [S, HID] partial output (its head-group's contribution
through Wo); the host sums the two partials per batch.

v2 design (vs the PE-transpose baseline):
  - Host pre-transposes X to X^T [HID, S] and casts all matmul operands
    to bf16: no PE transposes (was 384 matmuls + 96 DVE evacuations),
    half the input DMA bytes, and bf16 avoids fp32r's small-N matmul
    penalty on the causal-diagonal blocks.
  - Projections read X^T s-blocks of 512 straight from DRAM:
    q^T/k^T [e,s] via W-chunk stationary x X^T moving; v [s,e] natural
    via X^T-chunk stationary x W moving (with a ones column per head so
    the PV matmul also emits softmax denominators).
  - Attention per (512-q window j, head-pair hp) in [k,q] orientation:
    logits^T in PSUM, one strided exp per k-chunk covering both heads'
    valid columns only (no junk columns), causal mask applied as a
    post-exp multiplicative 0/1 triangle on bf16 SBUF (4x DVE mode)
    instead of an additive -1e9 on PSUM.
  - Normalize: copy the PSUM denominator row to SBUF, reciprocal there
    (a cross-partition-shift PSUM reciprocal mis-executes on HW), GpSimd
    partition_broadcast, one DVE multiply into bf16 ctx^T; the two
    heads' chains interleave so copies overlap the pool broadcasts.
  - Output projection ctx^T.T @ Wo per q-chunk.  Out-proj and remaining
    projection matmuls are emitted as single-matmul closures spread
    through the (ACT-bound) late attention windows so the in-order PE
    always has queued work while exp runs; the final four q-chunks are
    staged dv0-2 vs dv3 so only the last matmuls wait on the final
    normalize.  Staging copies ride DVE (ACT is the second-busiest
    engine); output DMAs go per 512-column half.
  - A "salt" input whose shape hashes this file works around the Neuron
    compile cache keying on the HLO only (the bass BIR rides in the
    custom_call backend_config, so kernels with identical I/O collide).
"""

import numpy as np

B, S, HID = 4, 2048, 1024
H_LOCAL, E_LOCAL = 8, 512  # heads / dk columns handled per core
N_CORES = 8

_cached = {}


def _salt_width():
    """Hash of this file's source, used to perturb an (unused) input
    tensor's shape.  The Neuron compile cache keys on the HLO module,
    which does NOT include the bass BIR payload (it rides in the
    custom_call backend_config) — without this, two different kernel
    builds with identical I/O signatures collide on one cache entry and
    a stale NEFF gets executed."""
    import hashlib

    with open(__file__, "rb") as f:
        h = hashlib.sha256(f.read()).hexdigest()
    return 2 + int(h, 16) % 4093


def _build():
    from concourse import bacc
    import concourse.bass as bass
    import concourse.mybir as mybir
    import concourse.tile as tile

    F32 = mybir.dt.float32
    BF16 = mybir.dt.bfloat16
    Exp = mybir.ActivationFunctionType.Exp

    nc = bacc.Bacc()
    nc.dram_tensor("salt", [1, _salt_width()], mybir.dt.uint8,
                   kind="ExternalInput")
    xq = nc.dram_tensor("xq", [HID, S], BF16, kind="ExternalInput")
    xk = nc.dram_tensor("xk", [HID, S], BF16, kind="ExternalInput")
    xv = nc.dram_tensor("xv", [HID, S], BF16, kind="ExternalInput")
    wq = nc.dram_tensor("wq", [HID, E_LOCAL], BF16, kind="ExternalInput")
    wk = nc.dram_tensor("wk", [HID, E_LOCAL], BF16, kind="ExternalInput")
    wv = nc.dram_tensor("wv", [HID, E_LOCAL], BF16, kind="ExternalInput")
    wo = nc.dram_tensor("wo", [E_LOCAL, HID], BF16, kind="ExternalInput")
    out = nc.dram_tensor("out", [S, HID], F32, kind="ExternalOutput")

    NDC = HID // 128       # 8 d-chunks
    NEC = E_LOCAL // 128   # 4 e-chunks = head pairs
    NKC = S // 128         # 16 k-chunks
    NSB = 4                # s-blocks of 512 (== q windows)
    SBW = S // NSB         # 512

    with tile.TileContext(nc) as tc:
        with (
            tc.sbuf_pool(name="consts", bufs=1) as consts,
            tc.sbuf_pool(name="persist", bufs=1) as persist,
            tc.sbuf_pool(name="stream", bufs=1) as sm,
            tc.psum_pool(name="ps", bufs=1) as ps,
        ):
            # multiplicative causal mask for diagonal [k,q] blocks:
            # 1 where k <= q else 0
            trif = consts.tile([128, 128], F32)
            nc.gpsimd.memset(trif, 1.0)
            nc.gpsimd.affine_select(
                out=trif, in_=trif,
                compare_op=mybir.AluOpType.is_ge, fill=0.0, base=0,
                pattern=[[1, 128]], channel_multiplier=-1,
            )
            trimask = consts.tile([128, 128], BF16)
            nc.vector.tensor_copy(trimask, trif)
            # warm the exp table while initial DMAs run
            warm = consts.tile([1, 2], F32)
            nc.vector.memset(warm, 0.0)
            nc.scalar.activation(warm, warm, Exp)

            kt_sb = [persist.tile([128, S], BF16, name=f"kt{i}", tag=f"kt{i}")
                     for i in range(NEC)]
            v_sb = [persist.tile([128, H_LOCAL, 65], BF16, name=f"v{i}",
                                 tag=f"v{i}") for i in range(NKC)]
            for i in range(NKC):
                nc.vector.memset(v_sb[i][:, :, 64:65], 1.0)

            wq_sb = sm.tile([128, NDC, E_LOCAL], BF16, tag="wq", bufs=1)
            wk_sb = sm.tile([128, NDC, E_LOCAL], BF16, tag="wk", bufs=1)
            wv_sb = sm.tile([128, NDC, E_LOCAL], BF16, tag="wv", bufs=1)
            wo_sb = sm.tile([128, NEC, HID], BF16, tag="wo", bufs=1)
            w_dram = {"q": wq, "k": wk, "v": wv}
            w_sb = {"q": wq_sb, "k": wk_sb, "v": wv_sb}

            qt_rot = {}   # (window, ec) -> [128, 512] bf16 q^T tile
            ctx_rot = {}  # (window, hp) -> [128, 512] bf16 ctx^T tile

            WORK_BUFS = 4  # unified PSUM ring: proj/out tiles + cpx pairs

            def work_tile(name):
                return ps.tile([128, 512], F32, tag="work", bufs=WORK_BUFS,
                               name=name)

            def proj_closures(sb, tname, dma_thunks=None):
                """Emit the X^T s-block DMA now (or hand quarter-DMA
                thunks to the caller for interleaving); return
                single-matmul closures for the projection."""
                s0 = sb * SBW
                xdram = {"q": xq, "k": xk, "v": xv}[tname]
                xt = sm.tile([128, NDC, SBW], BF16, tag="xt", bufs=2,
                             name=f"xt_{tname}{sb}")
                xsrc = xdram[:, s0:s0 + SBW].rearrange(
                    "(dc p) s -> p dc s", p=128)
                if dma_thunks is not None:
                    step = NDC // 4
                    for i in range(0, NDC, step):
                        dma_thunks.append(
                            lambda i=i: nc.sync.dma_start(
                                out=xt[:, i:i + step, :],
                                in_=xsrc[:, i:i + step, :]))
                else:
                    nc.sync.dma_start(out=xt, in_=xsrc)
                closures = []
                state = {}

                def mk_qk(ec, dc, w_s):
                    def go():
                        if dc == 0:
                            state[ec] = work_tile(f"pj{tname}_{sb}_{ec}")
                        nc.tensor.matmul(
                            state[ec], w_s[:, dc, ec * 128:(ec + 1) * 128],
                            xt[:, dc, :],
                            start=(dc == 0), stop=(dc == NDC - 1))
                        if dc == NDC - 1:
                            if tname == "q":
                                qt_rot[(sb, ec)] = sm.tile(
                                    [128, SBW], BF16, tag=f"qtr{ec}", bufs=3,
                                    name=f"qtr{ec}_{sb}")
                                nc.vector.tensor_copy(
                                    qt_rot[(sb, ec)], state[ec])
                            else:
                                nc.vector.tensor_copy(
                                    kt_sb[ec][:, s0:s0 + SBW], state[ec])
                    return go

                def mk_v(sc, dc):
                    def go():
                        if dc == 0:
                            state[sc] = work_tile(f"pv_{sb}_{sc}")
                        nc.tensor.matmul(
                            state[sc], xt[:, dc, sc * 128:(sc + 1) * 128],
                            wv_sb[:, dc, :],
                            start=(dc == 0), stop=(dc == NDC - 1))
                        if dc == NDC - 1:
                            ci = sb * (SBW // 128) + sc
                            nc.vector.tensor_copy(
                                v_sb[ci][:, :, 0:64],
                                state[sc].rearrange("p (h e) -> p h e",
                                                    h=H_LOCAL))
                    return go

                # dc-major: all 4 accumulators start on the first d-half
                # so the split DMAs overlap 16 matmuls of lead time
                if tname in ("q", "k"):
                    w_s = wq_sb if tname == "q" else wk_sb
                    for dc in range(NDC):
                        for ec in range(NEC):
                            closures.append(mk_qk(ec, dc, w_s))
                else:
                    for dc in range(NDC):
                        for sc in range(SBW // 128):
                            closures.append(mk_v(sc, dc))
                return closures

            def out_closures(qc):
                """Single-matmul closures for one out-proj q-chunk; the
                staging copy + DMA ride on each nh group's last matmul."""
                j, m = qc // 4, qc % 4
                state = {}
                closures = []

                def mk(nh, dv, copy_eng):
                    def go():
                        if dv == 0:
                            state[nh] = work_tile(f"po{qc}_{nh}")
                        nc.tensor.matmul(
                            state[nh],
                            ctx_rot[(j, dv)][:, m * 128:(m + 1) * 128],
                            wo_sb[:, dv, nh * 512:(nh + 1) * 512],
                            start=(dv == 0), stop=(dv == NEC - 1))
                        if dv == NEC - 1:
                            osb = sm.tile([128, 512], F32, tag="osb", bufs=2,
                                          name=f"osb{qc}_{nh}")
                            if copy_eng == "act":
                                nc.scalar.copy(osb, state[nh])
                            else:
                                nc.vector.tensor_copy(osb, state[nh])
                            nc.sync.dma_start(
                                out=out[qc * 128:(qc + 1) * 128,
                                        nh * 512:(nh + 1) * 512],
                                in_=osb)
                    return go

                # windows 1-3 are ACT-bound in practice -> stage on DVE,
                # except the very last block whose copies land after the
                # final exps (ACT idle, DVE busy with the last normalize)
                copy_eng = "act" if qc == 11 else "dve"
                for nh in range(2):
                    for dv in range(NEC):
                        closures.append(mk(nh, dv, copy_eng))
                return closures

            def chunk_front(j, hp, c):
                """Logits matmuls + exp (+ diagonal mask) for one chunk;
                returns the pending-PV tuple."""
                vo = max(0, c * 128 - j * SBW)
                lg = ps.tile([128, 1024], F32, tag="lg", bufs=2,
                             name=f"lg{hp}_{j}_{c}")
                pt = sm.tile([128, 2, SBW], BF16, tag="pt", bufs=2,
                             name=f"pt{hp}_{j}_{c}")
                qt = qt_rot[(j, hp)]
                for hi in range(2):
                    nc.tensor.matmul(
                        lg[:, hi * 512 + vo:(hi + 1) * 512],
                        kt_sb[hp][hi * 64:(hi + 1) * 64,
                                  c * 128:(c + 1) * 128],
                        qt[hi * 64:(hi + 1) * 64, vo:512],
                        start=True, stop=True)
                lg3 = lg.rearrange("p (hh q) -> p hh q", hh=2)
                nc.scalar.activation(
                    pt[:, :, vo:512], lg3[:, :, vo:512], Exp)
                if c >= 4 * j:
                    m = c - 4 * j
                    blk = pt[:, :, m * 128:(m + 1) * 128]
                    tri_b = bass.AP(
                        tensor=trimask.tensor, offset=trimask.offset,
                        ap=[trimask.ap[0], [0, 2], trimask.ap[1]],
                    )
                    nc.vector.tensor_mul(blk, blk, tri_b)
                return (c, vo, pt)

            def attention_unit(j, hp, fillers=(), pro=None,
                               emit_next=None):
                nlast = 4 * j + 3
                nchunks = 4 * j + 4
                cpx = [work_tile(f"cpx{hp}_{j}_{hi}") for hi in range(2)]
                ctx_rot[(j, hp)] = sm.tile([128, SBW], BF16, tag=f"ctxr{hp}",
                                           bufs=2, name=f"ctxr{hp}_{j}")
                fillers = list(fillers)
                filled = 0

                def emit_pv(c, vo, pt, his=(0, 1), lo=None, hh=512):
                    if lo is None:
                        lo = vo
                    for hi in his:
                        nc.tensor.matmul(
                            cpx[hi][0:65, lo:hh],
                            v_sb[c][:, hp * 2 + hi, :],
                            pt[:, hi, lo:hh],
                            start=(c == 0), stop=(c == nlast))

                bcasts = {}

                def denom_chain(hi):
                    # copy-then-reciprocal: DVE reciprocal mis-executes on
                    # HW when given a cross-partition-shift PSUM source
                    # (CoreSim accepts it), so keep the recip in SBUF with
                    # no partition shift
                    bcast = sm.tile([64, SBW], F32, tag="bcast", bufs=4,
                                    name=f"bc{hp}_{j}_{hi}")
                    bcasts[hi] = bcast
                    nc.vector.tensor_copy(bcast[0:1, :], cpx[hi][64:65, :])
                    nc.vector.reciprocal_approx_fast(
                        out=bcast[0:1, :], in_=bcast[0:1, :])
                    nc.gpsimd.partition_broadcast(bcast, bcast[0:1, :])

                def norm_mul(hi):
                    nc.vector.tensor_mul(
                        ctx_rot[(j, hp)][hi * 64:(hi + 1) * 64, :],
                        cpx[hi][0:64, :], bcasts[hi])

                # software-pipelined: logits(c+1) is emitted before PV(c)
                # so the PE has queued matmuls while ACT runs exp(c);
                # chunk 0 of the unit was emitted by the PREVIOUS unit
                # (cross-unit pipelining) and arrives as `pro`
                pending = pro if pro is not None else chunk_front(j, hp, 0)
                # front-load two fillers right after chunk 0's logits so
                # the PE has queued work while the previous unit's
                # normalize chain frees cpx and exp(0) runs
                while filled < min(len(fillers), 2):
                    fillers[filled]()
                    filled += 1
                for c in range(1, nchunks):
                    front = chunk_front(j, hp, c)
                    emit_pv(*pending)
                    pending = front
                    # spread filler matmuls across the chunk stream,
                    # front-loading two so the PE has queued work while
                    # the previous unit's normalize chain frees cpx
                    want = min(len(fillers),
                               2 + len(fillers) * (c + 1) // nchunks)
                    while filled < want:
                        fillers[filled]()
                        filled += 1
                # the next unit's first logits+exp go out before our
                # final PVs so ACT never idles across the unit switch
                pro_next = emit_next() if emit_next is not None else None
                # last chunk: hi=0's denominator chain starts on
                # DVE/GpSimd while the PE still streams hi=1's final PV,
                # and hi=1's copy/recip run during hi=0's pool broadcast
                emit_pv(*pending, his=(0,))
                denom_chain(0)
                emit_pv(*pending, his=(1,))
                denom_chain(1)
                norm_mul(0)
                norm_mul(1)
                while filled < len(fillers):
                    fillers[filled]()
                    filled += 1
                return pro_next

            def tail_partial(qc):
                """Tail out-proj phase 1: dv 0..2 accumulate (ready as
                soon as window 3's first three head-pairs normalize) in
                the (free by now) lg PSUM pool."""
                m = qc % 4
                po = ps.tile([128, 1024], F32, tag="lg", bufs=2,
                             name=f"po{qc}")
                for dv in range(NEC - 1):
                    for nh in range(2):
                        nc.tensor.matmul(
                            po[:, nh * 512:(nh + 1) * 512],
                            ctx_rot[(3, dv)][:, m * 128:(m + 1) * 128],
                            wo_sb[:, dv, nh * 512:(nh + 1) * 512],
                            start=(dv == 0), stop=False)
                return po

            def tail_finish(qc, po):
                """Tail phase 2: the dv=3 matmuls (gated on the last
                normalize) plus staging copies on both engines + DMA."""
                m = qc % 4
                for nh in range(2):
                    nc.tensor.matmul(
                        po[:, nh * 512:(nh + 1) * 512],
                        ctx_rot[(3, NEC - 1)][:, m * 128:(m + 1) * 128],
                        wo_sb[:, NEC - 1, nh * 512:(nh + 1) * 512],
                        start=False, stop=True)
                osb = sm.tile([128, 1024], F32, tag="osbt", bufs=2,
                              name=f"osbt{qc}")
                nc.vector.tensor_copy(osb[:, 0:512], po[:, 0:512])
                nc.scalar.copy(osb[:, 512:1024], po[:, 512:1024])
                nc.sync.dma_start(
                    out=out[qc * 128:(qc + 1) * 128, 0:512],
                    in_=osb[:, 0:512])
                nc.sync.dma_start(
                    out=out[qc * 128:(qc + 1) * 128, 512:1024],
                    in_=osb[:, 512:1024])

            # weight DMAs interleave with first-block projections in
            # consumption order, split in halves, so the first matmul
            # starts as soon as the first half-weight + half-X^T land
            for t in ("q", "k", "v"):
                wsrc = w_dram[t].rearrange("(dc p) e -> p dc e", p=128)
                step = NDC // 4
                thunks = []
                cls = proj_closures(0, t, dma_thunks=thunks)
                for i in range(0, NDC, step):
                    nc.sync.dma_start(out=w_sb[t][:, i:i + step, :],
                                      in_=wsrc[:, i:i + step, :])
                    thunks[i // step]()
                for cl in cls:
                    cl()
            for t in ("q", "k", "v"):
                for cl in proj_closures(1, t):
                    cl()
            nc.sync.dma_start(
                out=wo_sb, in_=wo.rearrange("(dv p) n -> p dv n", p=128))

            # Emission = program order.  Out-proj blocks of finished
            # windows ride INTERLEAVED inside the ACT-bound late windows
            # (single matmuls spread across the chunk stream keep the PE
            # from draining while exp runs); projections ride post-unit
            # only (4 work-ring tiles interleaved would positionally wait
            # on the live cpx pair -> deadlock).  Out-blocks of window j
            # must finish during window j+1 (ctx tiles rotate, bufs=2).
            fills_inter = {
                (2, 0): ("o", 4), (2, 1): ("o", 5),
                (2, 2): ("o", 6), (2, 3): ("o", 7),
                (3, 0): ("o", 8), (3, 1): ("o", 9),
                (3, 2): ("o", 10), (3, 3): ("o", 11),
            }
            fills_post = {
                (0, 0): [("p", 2, "q")], (0, 1): [("p", 2, "k")],
                (0, 2): [("p", 2, "v")], (0, 3): [("p", 3, "q")],
                (1, 0): [("o", 0)], (1, 1): [("o", 1)],
                (1, 2): [("o", 2)], (1, 3): [("o", 3), ("p", 3, "k")],
                (2, 3): [("p", 3, "v")],
            }

            def make(item):
                if item[0] == "p":
                    return proj_closures(item[1], item[2])
                return out_closures(item[1])

            post_extra = []
            for j in range(4):
                for hp in range(NEC):
                    inter = make(fills_inter[(j, hp)]) \
                        if (j, hp) in fills_inter else ()
                    if (j, hp) == (3, 3):
                        # o11's second half becomes pre-tail PE work (it
                        # needs only window-2 ctx, unlike the tail blocks
                        # which wait on the final normalize)
                        inter, post_extra = list(inter[:4]), list(inter[4:])
                    attention_unit(j, hp, inter)
                    for item in fills_post.get((j, hp), []):
                        for cl in make(item):
                            cl()
            for cl in post_extra:
                cl()
            # 2-deep staggering (lg pool bufs=2): dv0-2 of the next block
            # runs while the previous finishes, so the PE has ready work
            # while the final normalize chain completes on DVE/GpSimd
            po_a = tail_partial(12)
            po_b = tail_partial(13)
            tail_finish(12, po_a)
            po_a = tail_partial(14)
            tail_finish(13, po_b)
            po_b = tail_partial(15)
            tail_finish(14, po_a)
            tail_finish(15, po_b)

    nc.compile()
    return nc


def _make_in_maps(queries, keys, values, Wq, Wk, Wv, Wo):
    import ml_dtypes

    bf16 = ml_dtypes.bfloat16
    scale = np.float32(0.125)  # (DK//H) ** -0.5, exact power of two
    xt = {}
    for b in range(B):
        xt[b] = (
            np.ascontiguousarray(queries[b].T).astype(bf16),
            np.ascontiguousarray(keys[b].T).astype(bf16),
            np.ascontiguousarray(values[b].T).astype(bf16),
        )
    salt = np.zeros((1, _salt_width()), np.uint8)
    in_maps = []
    for c in range(N_CORES):
        b, g = divmod(c, 2)
        sl = slice(g * E_LOCAL, (g + 1) * E_LOCAL)
        in_maps.append({
            "salt": salt,
            "xq": xt[b][0],
            "xk": xt[b][1],
            "xv": xt[b][2],
            "wq": np.ascontiguousarray(Wq[:, sl] * scale).astype(bf16),
            "wk": np.ascontiguousarray(Wk[:, sl]).astype(bf16),
            "wv": np.ascontiguousarray(Wv[:, sl]).astype(bf16),
            "wo": np.ascontiguousarray(Wo[sl, :]).astype(bf16),
        })
    return in_maps


def kernel(queries, keys, values, mask=None, Wq=None, Wk=None, Wv=None,
           Wo=None, **_ignored):
    from concourse.bass_utils import run_bass_kernel_spmd

    if "nc" not in _cached:
        _cached["nc"] = _build()
    nc = _cached["nc"]

    queries = np.asarray(queries, dtype=np.float32)
    keys = np.asarray(keys, dtype=np.float32)
    values = np.asarray(values, dtype=np.float32)
    in_maps = _make_in_maps(queries, keys, values,
                            np.asarray(Wq, dtype=np.float32),
                            np.asarray(Wk, dtype=np.float32),
                            np.asarray(Wv, dtype=np.float32),
                            np.asarray(Wo, dtype=np.float32))
    res = run_bass_kernel_spmd(nc, in_maps, core_ids=list(range(N_CORES)))
    outs = res.results
    full = np.empty((B, S, HID), np.float32)
    for b in range(B):
        full[b] = outs[2 * b]["out"] + outs[2 * b + 1]["out"]
    return full


# revision 56
# speedup vs baseline: 1.7062x; 1.0157x over previous
"""Multi-head causal attention kernel for Trainium2 (8 NeuronCores).

Problem: B=4, S=2048, HID=1024, H=16 heads (head_dim 64), causal mask,
fp32 I/O.  out = softmax(mask + (XqWq)(XkWk)^T/8) (XvWv) Wo

Sharding: 8 cores = 4 batches x 2 head-groups.  Core c handles batch
c//2 and heads (c%2)*8 .. +8 (dk slice of 512).  Each core computes a
full-shape [S, HID] partial output (its head-group's contribution
through Wo); the host sums the two partials per batch.

v2 design (vs the PE-transpose baseline):
  - Host pre-transposes X to X^T [HID, S] and casts all matmul operands
    to bf16: no PE transposes (was 384 matmuls + 96 DVE evacuations),
    half the input DMA bytes, and bf16 avoids fp32r's small-N matmul
    penalty on the causal-diagonal blocks.
  - Projections read X^T s-blocks of 512 straight from DRAM:
    q^T/k^T [e,s] via W-chunk stationary x X^T moving; v [s,e] natural
    via X^T-chunk stationary x W moving (with a ones column per head so
    the PV matmul also emits softmax denominators).
  - Attention per (512-q window j, head-pair hp) in [k,q] orientation:
    logits^T in PSUM, one strided exp per k-chunk covering both heads'
    valid columns only (no junk columns), causal mask applied as a
    post-exp multiplicative 0/1 triangle on bf16 SBUF (4x DVE mode)
    instead of an additive -1e9 on PSUM.
  - Normalize: copy the PSUM denominator row to SBUF, reciprocal there
    (a cross-partition-shift PSUM reciprocal mis-executes on HW), GpSimd
    partition_broadcast, one DVE multiply into bf16 ctx^T; the two
    heads' chains interleave so copies overlap the pool broadcasts.
  - Output projection ctx^T.T @ Wo per q-chunk.  Out-proj and remaining
    projection matmuls are emitted as single-matmul closures spread
    through the (ACT-bound) late attention windows so the in-order PE
    always has queued work while exp runs; the final four q-chunks are
    staged dv0-2 vs dv3 so only the last matmuls wait on the final
    normalize.  Staging copies ride DVE (ACT is the second-busiest
    engine); output DMAs go per 512-column half.
  - A "salt" input whose shape hashes this file works around the Neuron
    compile cache keying on the HLO only (the bass BIR rides in the
    custom_call backend_config, so kernels with identical I/O collide).
"""

import numpy as np

B, S, HID = 4, 2048, 1024
H_LOCAL, E_LOCAL = 8, 512  # heads / dk columns handled per core
N_CORES = 8

_cached = {}


def _salt_width():
    """Hash of this file's source, used to perturb an (unused) input
    tensor's shape.  The Neuron compile cache keys on the HLO module,
    which does NOT include the bass BIR payload (it rides in the
    custom_call backend_config) — without this, two different kernel
    builds with identical I/O signatures collide on one cache entry and
    a stale NEFF gets executed."""
    import hashlib

    with open(__file__, "rb") as f:
        h = hashlib.sha256(f.read()).hexdigest()
    return 2 + int(h, 16) % 4093


def _build():
    from concourse import bacc
    import concourse.bass as bass
    import concourse.mybir as mybir
    import concourse.tile as tile

    F32 = mybir.dt.float32
    BF16 = mybir.dt.bfloat16
    Exp = mybir.ActivationFunctionType.Exp

    nc = bacc.Bacc()
    nc.dram_tensor("salt", [1, _salt_width()], mybir.dt.uint8,
                   kind="ExternalInput")
    xq = nc.dram_tensor("xq", [HID, S], BF16, kind="ExternalInput")
    xk = nc.dram_tensor("xk", [HID, S], BF16, kind="ExternalInput")
    xv = nc.dram_tensor("xv", [HID, S], BF16, kind="ExternalInput")
    wq = nc.dram_tensor("wq", [HID, E_LOCAL], BF16, kind="ExternalInput")
    wk = nc.dram_tensor("wk", [HID, E_LOCAL], BF16, kind="ExternalInput")
    wv = nc.dram_tensor("wv", [HID, E_LOCAL], BF16, kind="ExternalInput")
    wo = nc.dram_tensor("wo", [E_LOCAL, HID], BF16, kind="ExternalInput")
    out = nc.dram_tensor("out", [S, HID], F32, kind="ExternalOutput")

    NDC = HID // 128       # 8 d-chunks
    NEC = E_LOCAL // 128   # 4 e-chunks = head pairs
    NKC = S // 128         # 16 k-chunks
    NSB = 4                # s-blocks of 512 (== q windows)
    SBW = S // NSB         # 512

    with tile.TileContext(nc) as tc:
        with (
            tc.sbuf_pool(name="consts", bufs=1) as consts,
            tc.sbuf_pool(name="persist", bufs=1) as persist,
            tc.sbuf_pool(name="stream", bufs=1) as sm,
            tc.psum_pool(name="ps", bufs=1) as ps,
        ):
            # multiplicative causal mask for diagonal [k,q] blocks:
            # 1 where k <= q else 0
            trif = consts.tile([128, 128], F32)
            nc.gpsimd.memset(trif, 1.0)
            nc.gpsimd.affine_select(
                out=trif, in_=trif,
                compare_op=mybir.AluOpType.is_ge, fill=0.0, base=0,
                pattern=[[1, 128]], channel_multiplier=-1,
            )
            trimask = consts.tile([128, 128], BF16)
            nc.vector.tensor_copy(trimask, trif)
            # warm the exp table while initial DMAs run
            warm = consts.tile([1, 2], F32)
            nc.vector.memset(warm, 0.0)
            nc.scalar.activation(warm, warm, Exp)

            kt_sb = [persist.tile([128, S], BF16, name=f"kt{i}", tag=f"kt{i}")
                     for i in range(NEC)]
            v_sb = [persist.tile([128, H_LOCAL, 65], BF16, name=f"v{i}",
                                 tag=f"v{i}") for i in range(NKC)]
            for i in range(NKC):
                nc.vector.memset(v_sb[i][:, :, 64:65], 1.0)

            wq_sb = sm.tile([128, NDC, E_LOCAL], BF16, tag="wq", bufs=1)
            wk_sb = sm.tile([128, NDC, E_LOCAL], BF16, tag="wk", bufs=1)
            wv_sb = sm.tile([128, NDC, E_LOCAL], BF16, tag="wv", bufs=1)
            wo_sb = sm.tile([128, NEC, HID], BF16, tag="wo", bufs=1)
            w_dram = {"q": wq, "k": wk, "v": wv}
            w_sb = {"q": wq_sb, "k": wk_sb, "v": wv_sb}

            qt_rot = {}   # (window, ec) -> [128, 512] bf16 q^T tile
            ctx_rot = {}  # (window, hp) -> [128, 512] bf16 ctx^T tile

            WORK_BUFS = 4  # unified PSUM ring: proj/out tiles + cpx pairs

            def work_tile(name):
                return ps.tile([128, 512], F32, tag="work", bufs=WORK_BUFS,
                               name=name)

            def proj_closures(sb, tname, dma_thunks=None):
                """Emit the X^T s-block DMA now (or hand quarter-DMA
                thunks to the caller for interleaving); return
                single-matmul closures for the projection."""
                s0 = sb * SBW
                xdram = {"q": xq, "k": xk, "v": xv}[tname]
                xt = sm.tile([128, NDC, SBW], BF16, tag="xt", bufs=2,
                             name=f"xt_{tname}{sb}")
                xsrc = xdram[:, s0:s0 + SBW].rearrange(
                    "(dc p) s -> p dc s", p=128)
                if dma_thunks is not None:
                    step = NDC // 4
                    for i in range(0, NDC, step):
                        dma_thunks.append(
                            lambda i=i: nc.sync.dma_start(
                                out=xt[:, i:i + step, :],
                                in_=xsrc[:, i:i + step, :]))
                else:
                    nc.sync.dma_start(out=xt, in_=xsrc)
                closures = []
                state = {}

                def mk_qk(ec, dc, w_s):
                    def go():
                        if dc == 0:
                            state[ec] = work_tile(f"pj{tname}_{sb}_{ec}")
                        nc.tensor.matmul(
                            state[ec], w_s[:, dc, ec * 128:(ec + 1) * 128],
                            xt[:, dc, :],
                            start=(dc == 0), stop=(dc == NDC - 1))
                        if dc == NDC - 1:
                            if tname == "q":
                                qt_rot[(sb, ec)] = sm.tile(
                                    [128, SBW], BF16, tag=f"qtr{ec}", bufs=3,
                                    name=f"qtr{ec}_{sb}")
                                nc.vector.tensor_copy(
                                    qt_rot[(sb, ec)], state[ec])
                            else:
                                nc.vector.tensor_copy(
                                    kt_sb[ec][:, s0:s0 + SBW], state[ec])
                    return go

                def mk_v(sc, dc):
                    def go():
                        if dc == 0:
                            state[sc] = work_tile(f"pv_{sb}_{sc}")
                        nc.tensor.matmul(
                            state[sc], xt[:, dc, sc * 128:(sc + 1) * 128],
                            wv_sb[:, dc, :],
                            start=(dc == 0), stop=(dc == NDC - 1))
                        if dc == NDC - 1:
                            ci = sb * (SBW // 128) + sc
                            nc.vector.tensor_copy(
                                v_sb[ci][:, :, 0:64],
                                state[sc].rearrange("p (h e) -> p h e",
                                                    h=H_LOCAL))
                    return go

                # dc-major: all 4 accumulators start on the first d-half
                # so the split DMAs overlap 16 matmuls of lead time
                if tname in ("q", "k"):
                    w_s = wq_sb if tname == "q" else wk_sb
                    for dc in range(NDC):
                        for ec in range(NEC):
                            closures.append(mk_qk(ec, dc, w_s))
                else:
                    for dc in range(NDC):
                        for sc in range(SBW // 128):
                            closures.append(mk_v(sc, dc))
                return closures

            def out_closures(qc):
                """Single-matmul closures for one out-proj q-chunk; the
                staging copy + DMA ride on each nh group's last matmul."""
                j, m = qc // 4, qc % 4
                state = {}
                closures = []

                def mk(nh, dv, copy_eng):
                    def go():
                        if dv == 0:
                            state[nh] = work_tile(f"po{qc}_{nh}")
                        nc.tensor.matmul(
                            state[nh],
                            ctx_rot[(j, dv)][:, m * 128:(m + 1) * 128],
                            wo_sb[:, dv, nh * 512:(nh + 1) * 512],
                            start=(dv == 0), stop=(dv == NEC - 1))
                        if dv == NEC - 1:
                            osb = sm.tile([128, 512], F32, tag="osb", bufs=2,
                                          name=f"osb{qc}_{nh}")
                            if copy_eng == "act":
                                nc.scalar.copy(osb, state[nh])
                            else:
                                nc.vector.tensor_copy(osb, state[nh])
                            nc.sync.dma_start(
                                out=out[qc * 128:(qc + 1) * 128,
                                        nh * 512:(nh + 1) * 512],
                                in_=osb)
                    return go

                # windows 1-3 are ACT-bound in practice -> stage on DVE,
                # except the very last block whose copies land after the
                # final exps (ACT idle, DVE busy with the last normalize)
                copy_eng = "act" if qc == 11 else "dve"
                for nh in range(2):
                    for dv in range(NEC):
                        closures.append(mk(nh, dv, copy_eng))
                return closures

            def chunk_front(j, hp, c):
                """Logits matmuls + exp (+ diagonal mask) for one chunk;
                returns the pending-PV tuple."""
                vo = max(0, c * 128 - j * SBW)
                lg = ps.tile([128, 1024], F32, tag="lg", bufs=2,
                             name=f"lg{hp}_{j}_{c}")
                pt = sm.tile([128, 2, SBW], BF16, tag="pt", bufs=2,
                             name=f"pt{hp}_{j}_{c}")
                qt = qt_rot[(j, hp)]
                for hi in range(2):
                    nc.tensor.matmul(
                        lg[:, hi * 512 + vo:(hi + 1) * 512],
                        kt_sb[hp][hi * 64:(hi + 1) * 64,
                                  c * 128:(c + 1) * 128],
                        qt[hi * 64:(hi + 1) * 64, vo:512],
                        start=True, stop=True)
                lg3 = lg.rearrange("p (hh q) -> p hh q", hh=2)
                nc.scalar.activation(
                    pt[:, :, vo:512], lg3[:, :, vo:512], Exp)
                if c >= 4 * j:
                    m = c - 4 * j
                    blk = pt[:, :, m * 128:(m + 1) * 128]
                    tri_b = bass.AP(
                        tensor=trimask.tensor, offset=trimask.offset,
                        ap=[trimask.ap[0], [0, 2], trimask.ap[1]],
                    )
                    nc.vector.tensor_mul(blk, blk, tri_b)
                return (c, vo, pt)

            def attention_unit(j, hp, fillers=(), pro=None,
                               emit_next=None):
                nlast = 4 * j + 3
                nchunks = 4 * j + 4
                cpx = [work_tile(f"cpx{hp}_{j}_{hi}") for hi in range(2)]
                ctx_rot[(j, hp)] = sm.tile([128, SBW], BF16, tag=f"ctxr{hp}",
                                           bufs=2, name=f"ctxr{hp}_{j}")
                fillers = list(fillers)
                filled = 0

                def emit_pv(c, vo, pt, his=(0, 1), lo=None, hh=512):
                    if lo is None:
                        lo = vo
                    for hi in his:
                        nc.tensor.matmul(
                            cpx[hi][0:65, lo:hh],
                            v_sb[c][:, hp * 2 + hi, :],
                            pt[:, hi, lo:hh],
                            start=(c == 0), stop=(c == nlast))

                bcasts = {}

                def denom_chain(hi):
                    # copy-then-reciprocal: DVE reciprocal mis-executes on
                    # HW when given a cross-partition-shift PSUM source
                    # (CoreSim accepts it), so keep the recip in SBUF with
                    # no partition shift
                    bcast = sm.tile([64, SBW], F32, tag="bcast", bufs=2,
                                    name=f"bc{hp}_{j}_{hi}")
                    bcasts[hi] = bcast
                    nc.vector.tensor_copy(bcast[0:1, :], cpx[hi][64:65, :])
                    nc.vector.reciprocal_approx_fast(
                        out=bcast[0:1, :], in_=bcast[0:1, :])
                    nc.gpsimd.partition_broadcast(bcast, bcast[0:1, :])

                def norm_mul(hi):
                    nc.vector.tensor_mul(
                        ctx_rot[(j, hp)][hi * 64:(hi + 1) * 64, :],
                        cpx[hi][0:64, :], bcasts[hi])

                # software-pipelined: logits(c+1) is emitted before PV(c)
                # so the PE has queued matmuls while ACT runs exp(c);
                # chunk 0 of the unit was emitted by the PREVIOUS unit
                # (cross-unit pipelining) and arrives as `pro`
                pending = pro if pro is not None else chunk_front(j, hp, 0)
                # front-load two fillers right after chunk 0's logits so
                # the PE has queued work while the previous unit's
                # normalize chain frees cpx and exp(0) runs
                while filled < min(len(fillers), 2):
                    fillers[filled]()
                    filled += 1
                for c in range(1, nchunks):
                    front = chunk_front(j, hp, c)
                    emit_pv(*pending)
                    pending = front
                    # spread filler matmuls across the chunk stream,
                    # front-loading two so the PE has queued work while
                    # the previous unit's normalize chain frees cpx
                    want = min(len(fillers),
                               2 + len(fillers) * (c + 1) // nchunks)
                    while filled < want:
                        fillers[filled]()
                        filled += 1
                # the next unit's first logits+exp go out before our
                # final PVs so ACT never idles across the unit switch
                pro_next = emit_next() if emit_next is not None else None
                # last chunk: hi=0's denominator chain starts on
                # DVE/GpSimd while the PE still streams hi=1's final PV,
                # and hi=1's copy/recip run during hi=0's pool broadcast
                emit_pv(*pending, his=(0,))
                denom_chain(0)
                emit_pv(*pending, his=(1,))
                denom_chain(1)
                norm_mul(0)
                norm_mul(1)
                while filled < len(fillers):
                    fillers[filled]()
                    filled += 1
                return pro_next

            def tail_partial(qc):
                """Tail out-proj phase 1: dv 0..2 accumulate (ready as
                soon as window 3's first three head-pairs normalize) in
                the (free by now) lg PSUM pool."""
                m = qc % 4
                po = ps.tile([128, 1024], F32, tag="lg", bufs=2,
                             name=f"po{qc}")
                for dv in range(NEC - 1):
                    for nh in range(2):
                        nc.tensor.matmul(
                            po[:, nh * 512:(nh + 1) * 512],
                            ctx_rot[(3, dv)][:, m * 128:(m + 1) * 128],
                            wo_sb[:, dv, nh * 512:(nh + 1) * 512],
                            start=(dv == 0), stop=False)
                return po

            def tail_finish(qc, po):
                """Tail phase 2: the dv=3 matmuls (gated on the last
                normalize) plus staging copies on both engines + DMA."""
                m = qc % 4
                for nh in range(2):
                    nc.tensor.matmul(
                        po[:, nh * 512:(nh + 1) * 512],
                        ctx_rot[(3, NEC - 1)][:, m * 128:(m + 1) * 128],
                        wo_sb[:, NEC - 1, nh * 512:(nh + 1) * 512],
                        start=False, stop=True)
                osb = sm.tile([128, 1024], F32, tag="osbt", bufs=2,
                              name=f"osbt{qc}")
                nc.vector.tensor_copy(osb[:, 0:512], po[:, 0:512])
                nc.scalar.copy(osb[:, 512:1024], po[:, 512:1024])
                nc.sync.dma_start(
                    out=out[qc * 128:(qc + 1) * 128, 0:512],
                    in_=osb[:, 0:512])
                nc.sync.dma_start(
                    out=out[qc * 128:(qc + 1) * 128, 512:1024],
                    in_=osb[:, 512:1024])

            # weight DMAs interleave with first-block projections in
            # consumption order, split in halves, so the first matmul
            # starts as soon as the first half-weight + half-X^T land
            for t in ("q", "k", "v"):
                wsrc = w_dram[t].rearrange("(dc p) e -> p dc e", p=128)
                step = NDC // 4
                thunks = []
                cls = proj_closures(0, t, dma_thunks=thunks)
                for i in range(0, NDC, step):
                    nc.sync.dma_start(out=w_sb[t][:, i:i + step, :],
                                      in_=wsrc[:, i:i + step, :])
                    thunks[i // step]()
                for cl in cls:
                    cl()
            for t in ("q", "k", "v"):
                for cl in proj_closures(1, t):
                    cl()
            nc.sync.dma_start(
                out=wo_sb, in_=wo.rearrange("(dv p) n -> p dv n", p=128))

            # Emission = program order.  Out-proj blocks of finished
            # windows ride INTERLEAVED inside the ACT-bound late windows
            # (single matmuls spread across the chunk stream keep the PE
            # from draining while exp runs); projections ride post-unit
            # only (4 work-ring tiles interleaved would positionally wait
            # on the live cpx pair -> deadlock).  Out-blocks of window j
            # must finish during window j+1 (ctx tiles rotate, bufs=2).
            fills_inter = {
                (2, 0): ("o", 4), (2, 1): ("o", 5),
                (2, 2): ("o", 6), (2, 3): ("o", 7),
                (3, 0): ("o", 8), (3, 1): ("o", 9),
                (3, 2): ("o", 10), (3, 3): ("o", 11),
            }
            fills_post = {
                (0, 0): [("p", 2, "q")], (0, 1): [("p", 2, "k")],
                (0, 2): [("p", 2, "v")], (0, 3): [("p", 3, "q")],
                (1, 0): [("o", 0)], (1, 1): [("o", 1)],
                (1, 2): [("o", 2)], (1, 3): [("o", 3), ("p", 3, "k")],
                (2, 3): [("p", 3, "v")],
            }

            def make(item):
                if item[0] == "p":
                    return proj_closures(item[1], item[2])
                return out_closures(item[1])

            post_extra = []
            for j in range(4):
                for hp in range(NEC):
                    inter = make(fills_inter[(j, hp)]) \
                        if (j, hp) in fills_inter else ()
                    if (j, hp) == (3, 3):
                        # o11's second half becomes pre-tail PE work (it
                        # needs only window-2 ctx, unlike the tail blocks
                        # which wait on the final normalize)
                        inter, post_extra = list(inter[:4]), list(inter[4:])
                    attention_unit(j, hp, inter)
                    for item in fills_post.get((j, hp), []):
                        for cl in make(item):
                            cl()
            for cl in post_extra:
                cl()
            # 2-deep staggering (lg pool bufs=2): dv0-2 of the next block
            # runs while the previous finishes, so the PE has ready work
            # while the final normalize chain completes on DVE/GpSimd
            po_a = tail_partial(12)
            po_b = tail_partial(13)
            tail_finish(12, po_a)
            po_a = tail_partial(14)
            tail_finish(13, po_b)
            po_b = tail_partial(15)
            tail_finish(14, po_a)
            tail_finish(15, po_b)

    nc.compile()
    return nc


def _make_in_maps(queries, keys, values, Wq, Wk, Wv, Wo):
    import ml_dtypes

    bf16 = ml_dtypes.bfloat16
    scale = np.float32(0.125)  # (DK//H) ** -0.5, exact power of two
    xt = {}
    for b in range(B):
        xt[b] = (
            np.ascontiguousarray(queries[b].T).astype(bf16),
            np.ascontiguousarray(keys[b].T).astype(bf16),
            np.ascontiguousarray(values[b].T).astype(bf16),
        )
    salt = np.zeros((1, _salt_width()), np.uint8)
    in_maps = []
    for c in range(N_CORES):
        b, g = divmod(c, 2)
        sl = slice(g * E_LOCAL, (g + 1) * E_LOCAL)
        in_maps.append({
            "salt": salt,
            "xq": xt[b][0],
            "xk": xt[b][1],
            "xv": xt[b][2],
            "wq": np.ascontiguousarray(Wq[:, sl] * scale).astype(bf16),
            "wk": np.ascontiguousarray(Wk[:, sl]).astype(bf16),
            "wv": np.ascontiguousarray(Wv[:, sl]).astype(bf16),
            "wo": np.ascontiguousarray(Wo[sl, :]).astype(bf16),
        })
    return in_maps


def kernel(queries, keys, values, mask=None, Wq=None, Wk=None, Wv=None,
           Wo=None, **_ignored):
    from concourse.bass_utils import run_bass_kernel_spmd

    if "nc" not in _cached:
        _cached["nc"] = _build()
    nc = _cached["nc"]

    queries = np.asarray(queries, dtype=np.float32)
    keys = np.asarray(keys, dtype=np.float32)
    values = np.asarray(values, dtype=np.float32)
    in_maps = _make_in_maps(queries, keys, values,
                            np.asarray(Wq, dtype=np.float32),
                            np.asarray(Wk, dtype=np.float32),
                            np.asarray(Wv, dtype=np.float32),
                            np.asarray(Wo, dtype=np.float32))
    res = run_bass_kernel_spmd(nc, in_maps, core_ids=list(range(N_CORES)))
    outs = res.results
    full = np.empty((B, S, HID), np.float32)
    for b in range(B):
        full[b] = outs[2 * b]["out"] + outs[2 * b + 1]["out"]
    return full


# revision 61
# speedup vs baseline: 1.7822x; 1.0445x over previous
"""Multi-head causal attention kernel for Trainium2 (8 NeuronCores).

Problem: B=4, S=2048, HID=1024, H=16 heads (head_dim 64), causal mask,
fp32 I/O.  out = softmax(mask + (XqWq)(XkWk)^T/8) (XvWv) Wo

Sharding: 8 cores = 4 batches x 2 head-groups.  Core c handles batch
c//2 and heads (c%2)*8 .. +8 (dk slice of 512).  Each core computes a
full-shape [S, HID] partial output (its head-group's contribution
through Wo); the host sums the two partials per batch.

v2 design (vs the PE-transpose baseline):
  - Host pre-transposes X to X^T [HID, S] and casts all matmul operands
    to bf16: no PE transposes (was 384 matmuls + 96 DVE evacuations),
    half the input DMA bytes, and bf16 avoids fp32r's small-N matmul
    penalty on the causal-diagonal blocks.
  - Projections read X^T s-blocks of 512 straight from DRAM:
    q^T/k^T [e,s] via W-chunk stationary x X^T moving; v [s,e] natural
    via X^T-chunk stationary x W moving (with a ones column per head so
    the PV matmul also emits softmax denominators).
  - Attention per (512-q window j, head-pair hp) in [k,q] orientation:
    logits^T in PSUM, one strided exp per k-chunk covering both heads'
    valid columns only (no junk columns), causal mask applied as a
    post-exp multiplicative 0/1 triangle on bf16 SBUF (4x DVE mode)
    instead of an additive -1e9 on PSUM.
  - Normalize: copy the PSUM denominator row to SBUF, reciprocal there
    (a cross-partition-shift PSUM reciprocal mis-executes on HW), GpSimd
    partition_broadcast, one DVE multiply into bf16 ctx^T; the two
    heads' chains interleave so copies overlap the pool broadcasts.
  - Output projection ctx^T.T @ Wo per q-chunk.  Out-proj and remaining
    projection matmuls are emitted as single-matmul closures spread
    through the (ACT-bound) late attention windows so the in-order PE
    always has queued work while exp runs; the hi=1 PV stream lags one
    chunk behind hi=0 so a fresh unit's first PVs don't collide with
    the previous unit's normalize chain releasing its PSUM accumulators;
    the final four q-chunks are staged dv0-2 vs dv3 so only the last
    matmuls wait on the final normalize.  Staging copies ride DVE (ACT
    is the second-busiest engine); output DMAs go per 512-column half.
  - A "salt" input whose shape hashes this file works around the Neuron
    compile cache keying on the HLO only (the bass BIR rides in the
    custom_call backend_config, so kernels with identical I/O collide).
"""

import numpy as np

B, S, HID = 4, 2048, 1024
H_LOCAL, E_LOCAL = 8, 512  # heads / dk columns handled per core
N_CORES = 8

_cached = {}


def _salt_width():
    """Hash of this file's source, used to perturb an (unused) input
    tensor's shape.  The Neuron compile cache keys on the HLO module,
    which does NOT include the bass BIR payload (it rides in the
    custom_call backend_config) — without this, two different kernel
    builds with identical I/O signatures collide on one cache entry and
    a stale NEFF gets executed."""
    import hashlib

    with open(__file__, "rb") as f:
        h = hashlib.sha256(f.read()).hexdigest()
    return 2 + int(h, 16) % 4093


def _build():
    from concourse import bacc
    import concourse.bass as bass
    import concourse.mybir as mybir
    import concourse.tile as tile

    F32 = mybir.dt.float32
    BF16 = mybir.dt.bfloat16
    Exp = mybir.ActivationFunctionType.Exp

    nc = bacc.Bacc()
    nc.dram_tensor("salt", [1, _salt_width()], mybir.dt.uint8,
                   kind="ExternalInput")
    xq = nc.dram_tensor("xq", [HID, S], BF16, kind="ExternalInput")
    xk = nc.dram_tensor("xk", [HID, S], BF16, kind="ExternalInput")
    xv = nc.dram_tensor("xv", [HID, S], BF16, kind="ExternalInput")
    wq = nc.dram_tensor("wq", [HID, E_LOCAL], BF16, kind="ExternalInput")
    wk = nc.dram_tensor("wk", [HID, E_LOCAL], BF16, kind="ExternalInput")
    wv = nc.dram_tensor("wv", [HID, E_LOCAL], BF16, kind="ExternalInput")
    wo = nc.dram_tensor("wo", [E_LOCAL, HID], BF16, kind="ExternalInput")
    out = nc.dram_tensor("out", [S, HID], F32, kind="ExternalOutput")

    NDC = HID // 128       # 8 d-chunks
    NEC = E_LOCAL // 128   # 4 e-chunks = head pairs
    NKC = S // 128         # 16 k-chunks
    NSB = 4                # s-blocks of 512 (== q windows)
    SBW = S // NSB         # 512

    with tile.TileContext(nc) as tc:
        with (
            tc.sbuf_pool(name="consts", bufs=1) as consts,
            tc.sbuf_pool(name="persist", bufs=1) as persist,
            tc.sbuf_pool(name="stream", bufs=1) as sm,
            tc.psum_pool(name="ps", bufs=1) as ps,
        ):
            # multiplicative causal mask for diagonal [k,q] blocks:
            # 1 where k <= q else 0
            trif = consts.tile([128, 128], F32)
            nc.gpsimd.memset(trif, 1.0)
            nc.gpsimd.affine_select(
                out=trif, in_=trif,
                compare_op=mybir.AluOpType.is_ge, fill=0.0, base=0,
                pattern=[[1, 128]], channel_multiplier=-1,
            )
            trimask = consts.tile([128, 128], BF16)
            nc.vector.tensor_copy(trimask, trif)
            # warm the exp table while initial DMAs run
            warm = consts.tile([1, 2], F32)
            nc.vector.memset(warm, 0.0)
            nc.scalar.activation(warm, warm, Exp)

            kt_sb = [persist.tile([128, S], BF16, name=f"kt{i}", tag=f"kt{i}")
                     for i in range(NEC)]
            v_sb = [persist.tile([128, H_LOCAL, 65], BF16, name=f"v{i}",
                                 tag=f"v{i}") for i in range(NKC)]
            for i in range(NKC):
                nc.vector.memset(v_sb[i][:, :, 64:65], 1.0)

            wq_sb = sm.tile([128, NDC, E_LOCAL], BF16, tag="wq", bufs=1)
            wk_sb = sm.tile([128, NDC, E_LOCAL], BF16, tag="wk", bufs=1)
            wv_sb = sm.tile([128, NDC, E_LOCAL], BF16, tag="wv", bufs=1)
            wo_sb = sm.tile([128, NEC, HID], BF16, tag="wo", bufs=1)
            w_dram = {"q": wq, "k": wk, "v": wv}
            w_sb = {"q": wq_sb, "k": wk_sb, "v": wv_sb}

            qt_rot = {}   # (window, ec) -> [128, 512] bf16 q^T tile
            ctx_rot = {}  # (window, hp) -> [128, 512] bf16 ctx^T tile

            WORK_BUFS = 4  # unified PSUM ring: proj/out tiles + cpx pairs

            def work_tile(name):
                return ps.tile([128, 512], F32, tag="work", bufs=WORK_BUFS,
                               name=name)

            def proj_closures(sb, tname, dma_thunks=None):
                """Emit the X^T s-block DMA now (or hand quarter-DMA
                thunks to the caller for interleaving); return
                single-matmul closures for the projection."""
                s0 = sb * SBW
                xdram = {"q": xq, "k": xk, "v": xv}[tname]
                xt = sm.tile([128, NDC, SBW], BF16, tag="xt", bufs=2,
                             name=f"xt_{tname}{sb}")
                xsrc = xdram[:, s0:s0 + SBW].rearrange(
                    "(dc p) s -> p dc s", p=128)
                if dma_thunks is not None:
                    step = NDC // 4
                    for i in range(0, NDC, step):
                        dma_thunks.append(
                            lambda i=i: nc.sync.dma_start(
                                out=xt[:, i:i + step, :],
                                in_=xsrc[:, i:i + step, :]))
                else:
                    nc.sync.dma_start(out=xt, in_=xsrc)
                closures = []
                state = {}

                def mk_qk(ec, dc, w_s):
                    def go():
                        if dc == 0:
                            state[ec] = work_tile(f"pj{tname}_{sb}_{ec}")
                        nc.tensor.matmul(
                            state[ec], w_s[:, dc, ec * 128:(ec + 1) * 128],
                            xt[:, dc, :],
                            start=(dc == 0), stop=(dc == NDC - 1))
                        if dc == NDC - 1:
                            if tname == "q":
                                qt_rot[(sb, ec)] = sm.tile(
                                    [128, SBW], BF16, tag=f"qtr{ec}", bufs=3,
                                    name=f"qtr{ec}_{sb}")
                                nc.vector.tensor_copy(
                                    qt_rot[(sb, ec)], state[ec])
                            else:
                                nc.vector.tensor_copy(
                                    kt_sb[ec][:, s0:s0 + SBW], state[ec])
                    return go

                def mk_v(sc, dc):
                    def go():
                        if dc == 0:
                            state[sc] = work_tile(f"pv_{sb}_{sc}")
                        nc.tensor.matmul(
                            state[sc], xt[:, dc, sc * 128:(sc + 1) * 128],
                            wv_sb[:, dc, :],
                            start=(dc == 0), stop=(dc == NDC - 1))
                        if dc == NDC - 1:
                            ci = sb * (SBW // 128) + sc
                            nc.vector.tensor_copy(
                                v_sb[ci][:, :, 0:64],
                                state[sc].rearrange("p (h e) -> p h e",
                                                    h=H_LOCAL))
                    return go

                # dc-major: all 4 accumulators start on the first d-half
                # so the split DMAs overlap 16 matmuls of lead time
                if tname in ("q", "k"):
                    w_s = wq_sb if tname == "q" else wk_sb
                    for dc in range(NDC):
                        for ec in range(NEC):
                            closures.append(mk_qk(ec, dc, w_s))
                else:
                    for dc in range(NDC):
                        for sc in range(SBW // 128):
                            closures.append(mk_v(sc, dc))
                return closures

            def out_closures(qc):
                """Single-matmul closures for one out-proj q-chunk; the
                staging copy + DMA ride on each nh group's last matmul."""
                j, m = qc // 4, qc % 4
                state = {}
                closures = []

                def mk(nh, dv, copy_eng):
                    def go():
                        if dv == 0:
                            state[nh] = work_tile(f"po{qc}_{nh}")
                        nc.tensor.matmul(
                            state[nh],
                            ctx_rot[(j, dv)][:, m * 128:(m + 1) * 128],
                            wo_sb[:, dv, nh * 512:(nh + 1) * 512],
                            start=(dv == 0), stop=(dv == NEC - 1))
                        if dv == NEC - 1:
                            osb = sm.tile([128, 512], F32, tag="osb", bufs=2,
                                          name=f"osb{qc}_{nh}")
                            if copy_eng == "act":
                                nc.scalar.copy(osb, state[nh])
                            else:
                                nc.vector.tensor_copy(osb, state[nh])
                            nc.sync.dma_start(
                                out=out[qc * 128:(qc + 1) * 128,
                                        nh * 512:(nh + 1) * 512],
                                in_=osb)
                    return go

                # windows 1-3 are ACT-bound in practice -> stage on DVE,
                # except the very last block whose copies land after the
                # final exps (ACT idle, DVE busy with the last normalize)
                copy_eng = "act" if qc == 11 else "dve"
                for nh in range(2):
                    for dv in range(NEC):
                        closures.append(mk(nh, dv, copy_eng))
                return closures

            def chunk_front(j, hp, c):
                """Logits matmuls + exp (+ diagonal mask) for one chunk;
                returns the pending-PV tuple."""
                vo = max(0, c * 128 - j * SBW)
                lg = ps.tile([128, 1024], F32, tag="lg", bufs=2,
                             name=f"lg{hp}_{j}_{c}")
                pt = sm.tile([128, 2, SBW], BF16, tag="pt", bufs=4,
                             name=f"pt{hp}_{j}_{c}")
                qt = qt_rot[(j, hp)]
                for hi in range(2):
                    nc.tensor.matmul(
                        lg[:, hi * 512 + vo:(hi + 1) * 512],
                        kt_sb[hp][hi * 64:(hi + 1) * 64,
                                  c * 128:(c + 1) * 128],
                        qt[hi * 64:(hi + 1) * 64, vo:512],
                        start=True, stop=True)
                lg3 = lg.rearrange("p (hh q) -> p hh q", hh=2)
                nc.scalar.activation(
                    pt[:, :, vo:512], lg3[:, :, vo:512], Exp)
                if c >= 4 * j:
                    m = c - 4 * j
                    blk = pt[:, :, m * 128:(m + 1) * 128]
                    tri_b = bass.AP(
                        tensor=trimask.tensor, offset=trimask.offset,
                        ap=[trimask.ap[0], [0, 2], trimask.ap[1]],
                    )
                    nc.vector.tensor_mul(blk, blk, tri_b)
                return (c, vo, pt)

            def attention_unit(j, hp, fillers=(), pro=None,
                               emit_next=None):
                nlast = 4 * j + 3
                nchunks = 4 * j + 4
                cpx = [work_tile(f"cpx{hp}_{j}_{hi}") for hi in range(2)]
                ctx_rot[(j, hp)] = sm.tile([128, SBW], BF16, tag=f"ctxr{hp}",
                                           bufs=3, name=f"ctxr{hp}_{j}")
                fillers = list(fillers)
                filled = 0

                def emit_pv(c, vo, pt, his=(0, 1), lo=None, hh=512):
                    if lo is None:
                        lo = vo
                    for hi in his:
                        nc.tensor.matmul(
                            cpx[hi][0:65, lo:hh],
                            v_sb[c][:, hp * 2 + hi, :],
                            pt[:, hi, lo:hh],
                            start=(c == 0), stop=(c == nlast))

                bcasts = {}

                def denom_chain(hi):
                    # copy-then-reciprocal: DVE reciprocal mis-executes on
                    # HW when given a cross-partition-shift PSUM source
                    # (CoreSim accepts it), so keep the recip in SBUF with
                    # no partition shift
                    bcast = sm.tile([64, SBW], F32, tag="bcast", bufs=2,
                                    name=f"bc{hp}_{j}_{hi}")
                    bcasts[hi] = bcast
                    nc.vector.tensor_copy(bcast[0:1, :], cpx[hi][64:65, :])
                    nc.vector.reciprocal_approx_fast(
                        out=bcast[0:1, :], in_=bcast[0:1, :])
                    nc.gpsimd.partition_broadcast(bcast, bcast[0:1, :])

                def norm_mul(hi):
                    nc.vector.tensor_mul(
                        ctx_rot[(j, hp)][hi * 64:(hi + 1) * 64, :],
                        cpx[hi][0:64, :], bcasts[hi])

                # software-pipelined: logits(c+1) is emitted before PV(c),
                # and the hi=1 PV stream lags one further chunk behind
                # hi=0 so the first PVs of this unit don't collide with
                # the previous unit's normalize chain freeing its cpx.
                # chunk 0 of the unit was emitted by the PREVIOUS unit
                # (cross-unit pipelining) and arrives as `pro`
                pend0 = pro if pro is not None else chunk_front(j, hp, 0)
                pend1 = None
                # front-load two fillers right after chunk 0's logits so
                # the PE has queued work while the previous unit's
                # normalize chain frees cpx and exp(0) runs
                while filled < min(len(fillers), 2):
                    fillers[filled]()
                    filled += 1
                for c in range(1, nchunks):
                    front = chunk_front(j, hp, c)
                    emit_pv(*pend0, his=(0,))
                    if pend1 is not None:
                        emit_pv(*pend1, his=(1,))
                    pend1 = pend0
                    pend0 = front
                    want = min(len(fillers),
                               2 + len(fillers) * (c + 1) // nchunks)
                    while filled < want:
                        fillers[filled]()
                        filled += 1
                # the next unit's first logits+exp go out before our
                # final PVs so ACT never idles across the unit switch
                pro_next = emit_next() if emit_next is not None else None
                # drain: hi=0's denominator chain starts on DVE/GpSimd
                # while the PE still streams the lagging hi=1 PVs
                emit_pv(*pend0, his=(0,))
                denom_chain(0)
                emit_pv(*pend1, his=(1,))
                emit_pv(*pend0, his=(1,))
                denom_chain(1)
                norm_mul(0)
                norm_mul(1)
                while filled < len(fillers):
                    fillers[filled]()
                    filled += 1
                return pro_next

            def tail_partial(qc):
                """Tail out-proj phase 1: dv 0..2 accumulate (ready as
                soon as window 3's first three head-pairs normalize) in
                the (free by now) lg PSUM pool."""
                m = qc % 4
                po = ps.tile([128, 1024], F32, tag="lg", bufs=2,
                             name=f"po{qc}")
                for dv in range(NEC - 1):
                    for nh in range(2):
                        nc.tensor.matmul(
                            po[:, nh * 512:(nh + 1) * 512],
                            ctx_rot[(3, dv)][:, m * 128:(m + 1) * 128],
                            wo_sb[:, dv, nh * 512:(nh + 1) * 512],
                            start=(dv == 0), stop=False)
                return po

            def tail_finish(qc, po):
                """Tail phase 2: the dv=3 matmuls (gated on the last
                normalize) plus staging copies on both engines + DMA."""
                m = qc % 4
                for nh in range(2):
                    nc.tensor.matmul(
                        po[:, nh * 512:(nh + 1) * 512],
                        ctx_rot[(3, NEC - 1)][:, m * 128:(m + 1) * 128],
                        wo_sb[:, NEC - 1, nh * 512:(nh + 1) * 512],
                        start=False, stop=True)
                osb = sm.tile([128, 1024], F32, tag="osbt", bufs=2,
                              name=f"osbt{qc}")
                nc.vector.tensor_copy(osb[:, 0:512], po[:, 0:512])
                nc.scalar.copy(osb[:, 512:1024], po[:, 512:1024])
                nc.sync.dma_start(
                    out=out[qc * 128:(qc + 1) * 128, 0:512],
                    in_=osb[:, 0:512])
                nc.sync.dma_start(
                    out=out[qc * 128:(qc + 1) * 128, 512:1024],
                    in_=osb[:, 512:1024])

            # weight DMAs interleave with first-block projections in
            # consumption order, split in halves, so the first matmul
            # starts as soon as the first half-weight + half-X^T land
            for t in ("q", "k", "v"):
                wsrc = w_dram[t].rearrange("(dc p) e -> p dc e", p=128)
                step = NDC // 4
                thunks = []
                cls = proj_closures(0, t, dma_thunks=thunks)
                for i in range(0, NDC, step):
                    nc.sync.dma_start(out=w_sb[t][:, i:i + step, :],
                                      in_=wsrc[:, i:i + step, :])
                    thunks[i // step]()
                for cl in cls:
                    cl()
            for t in ("q", "k", "v"):
                for cl in proj_closures(1, t):
                    cl()
            nc.sync.dma_start(
                out=wo_sb, in_=wo.rearrange("(dv p) n -> p dv n", p=128))

            # Emission = program order.  Out-proj blocks of finished
            # windows ride INTERLEAVED inside the ACT-bound late windows
            # (single matmuls spread across the chunk stream keep the PE
            # from draining while exp runs); projections ride post-unit
            # only (4 work-ring tiles interleaved would positionally wait
            # on the live cpx pair -> deadlock).  Out-blocks of window j
            # must finish during window j+1 (ctx tiles rotate, bufs=2).
            fills_inter = {
                (2, 0): ("o", 0), (2, 1): ("o", 1),
                (2, 2): ("o", 2), (2, 3): ("o", 3),
                (3, 0): ("o", 8), (3, 1): ("o", 9),
                (3, 2): ("o", 10), (3, 3): ("o", 11),
            }
            fills_post = {
                (0, 0): [("p", 2, "q")], (0, 1): [("p", 2, "k")],
                (0, 2): [("p", 2, "v")], (0, 3): [("p", 3, "q")],
                (1, 3): [("p", 3, "k")],
                (2, 3): [("p", 3, "v")],
                (3, 0): [("o", 4)], (3, 1): [("o", 5)],
                (3, 2): [("o", 6)], (3, 3): [("o", 7)],
            }

            def make(item):
                if item[0] == "p":
                    return proj_closures(item[1], item[2])
                return out_closures(item[1])

            post_extra = []
            for j in range(4):
                for hp in range(NEC):
                    inter = make(fills_inter[(j, hp)]) \
                        if (j, hp) in fills_inter else ()
                    if (j, hp) == (3, 3):
                        # o11's second half becomes pre-tail PE work (it
                        # needs only window-2 ctx, unlike the tail blocks
                        # which wait on the final normalize)
                        inter, post_extra = list(inter[:4]), list(inter[4:])
                    attention_unit(j, hp, inter)
                    for item in fills_post.get((j, hp), []):
                        for cl in make(item):
                            cl()
            for cl in post_extra:
                cl()
            # 2-deep staggering (lg pool bufs=2): dv0-2 of the next block
            # runs while the previous finishes, so the PE has ready work
            # while the final normalize chain completes on DVE/GpSimd
            po_a = tail_partial(12)
            po_b = tail_partial(13)
            tail_finish(12, po_a)
            po_a = tail_partial(14)
            tail_finish(13, po_b)
            po_b = tail_partial(15)
            tail_finish(14, po_a)
            tail_finish(15, po_b)

    nc.compile()
    return nc


def _make_in_maps(queries, keys, values, Wq, Wk, Wv, Wo):
    import ml_dtypes

    bf16 = ml_dtypes.bfloat16
    scale = np.float32(0.125)  # (DK//H) ** -0.5, exact power of two
    xt = {}
    for b in range(B):
        xt[b] = (
            np.ascontiguousarray(queries[b].T).astype(bf16),
            np.ascontiguousarray(keys[b].T).astype(bf16),
            np.ascontiguousarray(values[b].T).astype(bf16),
        )
    salt = np.zeros((1, _salt_width()), np.uint8)
    in_maps = []
    for c in range(N_CORES):
        b, g = divmod(c, 2)
        sl = slice(g * E_LOCAL, (g + 1) * E_LOCAL)
        in_maps.append({
            "salt": salt,
            "xq": xt[b][0],
            "xk": xt[b][1],
            "xv": xt[b][2],
            "wq": np.ascontiguousarray(Wq[:, sl] * scale).astype(bf16),
            "wk": np.ascontiguousarray(Wk[:, sl]).astype(bf16),
            "wv": np.ascontiguousarray(Wv[:, sl]).astype(bf16),
            "wo": np.ascontiguousarray(Wo[sl, :]).astype(bf16),
        })
    return in_maps


def kernel(queries, keys, values, mask=None, Wq=None, Wk=None, Wv=None,
           Wo=None, **_ignored):
    from concourse.bass_utils import run_bass_kernel_spmd

    if "nc" not in _cached:
        _cached["nc"] = _build()
    nc = _cached["nc"]

    queries = np.asarray(queries, dtype=np.float32)
    keys = np.asarray(keys, dtype=np.float32)
    values = np.asarray(values, dtype=np.float32)
    in_maps = _make_in_maps(queries, keys, values,
                            np.asarray(Wq, dtype=np.float32),
                            np.asarray(Wk, dtype=np.float32),
                            np.asarray(Wv, dtype=np.float32),
                            np.asarray(Wo, dtype=np.float32))
    res = run_bass_kernel_spmd(nc, in_maps, core_ids=list(range(N_CORES)))
    outs = res.results
    full = np.empty((B, S, HID), np.float32)
    for b in range(B):
        full[b] = outs[2 * b]["out"] + outs[2 * b + 1]["out"]
    return full
